# revision 20
# baseline (speedup 1.0000x reference)
"""Trainium2 Bass kernel for a 3-layer GCN bottleneck block (50k nodes, 800k edges).

Strategy (8 NeuronCores, dst-node sharding):
- Host: relabel nodes into 8 cores x TILES tiles x 128 slots, balancing per-tile
  in-degree. Edges sorted by (dst tile, src half, src id); each (tile, half)
  group padded to a uniform chunk count so one SPMD program serves all cores.
  Self-loops become plain edges.
- All three convs aggregate in 64-dim space (conv3 rewritten as (A~ @ a2) @ W3).
- Per layer: compute local h-shard = a @ W with dinv[src] folded in, AllGather
  the [NPAD, 64] fp32 table, dma_gather 256B rows per edge (src-sorted for HBM
  locality), segment-sum via one-hot matmul: aggT[64f,128d] += M16.T @ S where
  S = is_equal(iota, dstloc) built in one DVE pass per batch (pads get dstloc=-1
  so their S column is zero). dinv[dst] applied during PSUM eviction.
- BatchNorm: per-core feature-major partial sums, tiny AllReduce, ACT-fused
  scale/bias/relu. Final layer: W3 matmul to 256-dim, BN3, residual, relu,
  output transposed; host untransposes/unpermutes.
"""

import os
import numpy as np

DIN = 256
DOUT = 64
EPS = 1e-5
NCORE = 8

# default (real-problem) geometry; test_sim.py overrides via configure()
_CFG = {}


def configure(N, E, NLOC, BTILES):
    TILES = (NLOC + 127) // 128
    NP = TILES * 128
    TL = (TILES + 1) // 2                     # lo tiles 0..TL-1
    SZ_LO, SZ_HI = TL * 128, NP - TL * 128
    _CFG.update(
        N=N, E=E, NLOC=NLOC, TILES=TILES, NP=NP,
        NPAD=NCORE * NP,
        TL=TL, SZ_LO=SZ_LO, SZ_HI=SZ_HI,
        HALF_LO=NCORE * SZ_LO, HALF_HI=NCORE * SZ_HI,
        LASTV=NLOC - 128 * (TILES - 1),       # valid rows in last tile
        BTILES=BTILES,
    )


configure(N=50000, E=800000, NLOC=6250, BTILES=3)

LAST_EXEC_NS = None    # set by kernel() when GCN_TRACE=1


# ----------------------------------------------------------------------------
# host-side graph prep
# ----------------------------------------------------------------------------

def _prep_graph(ei):
    import heapq
    N, TILES, NP = _CFG["N"], _CFG["TILES"], _CFG["NP"]
    LASTV = _CFG["LASTV"]
    src, dst = ei[0].astype(np.int64), ei[1].astype(np.int64)
    deg = np.bincount(dst, minlength=N).astype(np.float32) + 1.0
    dinv = (1.0 / np.sqrt(deg)).astype(np.float32)
    indeg = np.bincount(dst, minlength=N)

    # Balance BOTH lo and hi in-edge loads per (core,tile) slot: the padded
    # chunk counts K_lo/K_hi are set by the max over slots, so minimize the
    # larger of the two running loads. lo/hi of an edge depends on the SOURCE
    # node's final core, unknown during assignment — approximate with a
    # first-pass assignment by total degree, then rebalance on realized lo/hi.
    nslot = NCORE * TILES
    cap = np.full(nslot, 128, np.int64)
    cap[TILES - 1 :: TILES] = LASTV

    def greedy(key_lo, key_hi):
        order = np.argsort(-(key_lo + key_hi), kind="stable")
        load_lo = np.zeros(nslot, np.float64)
        load_hi = np.zeros(nslot, np.float64)
        fill = np.zeros(nslot, np.int64)
        slot_of = np.empty(N, np.int64)
        col_of = np.empty(N, np.int64)
        heap = [(0.0, s) for s in range(nslot)]
        heapq.heapify(heap)
        for n in order:
            while True:
                l, s = heapq.heappop(heap)
                if fill[s] < cap[s] and l == max(load_lo[s], load_hi[s]):
                    break
                if fill[s] < cap[s]:
                    heapq.heappush(heap, (max(load_lo[s], load_hi[s]), s))
            slot_of[n] = s
            col_of[n] = fill[s]
            fill[s] += 1
            load_lo[s] += key_lo[n]
            load_hi[s] += key_hi[n]
            if fill[s] < cap[s]:
                heapq.heappush(heap, (max(load_lo[s], load_hi[s]), s))
        return slot_of, col_of

    # pass 1: split unknown -> assume half/half
    half = indeg.astype(np.float64) / 2.0
    slot_of, col_of = greedy(half, half)
    # refine: realized lo/hi per dst node under previous pass's tiles
    best = None
    for _ in range(6):
        tile1 = slot_of % TILES
        src_hi = tile1[src] >= _CFG["TL"]
        lo_cnt = np.bincount(dst[~src_hi], minlength=N).astype(np.float64)
        hi_cnt = np.bincount(dst[src_hi], minlength=N).astype(np.float64)
        slot_of, col_of = greedy(lo_cnt, hi_cnt)
        # realized max per (slot, half) under THIS assignment
        tile2 = slot_of % TILES
        s_hi = tile2[src] >= _CFG["TL"]
        e_slot = slot_of[dst]
        sl_self_lo = (np.arange(N) % 1 == 0)  # self loop: src tile == own tile
        lo_l = np.bincount(e_slot[~s_hi], minlength=nslot).astype(np.int64)
        hi_l = np.bincount(e_slot[s_hi], minlength=nslot).astype(np.int64)
        mx = max(lo_l.max(), hi_l.max())
        if best is None or mx < best[0]:
            best = (mx, slot_of.copy(), col_of.copy())
        if mx <= 9 * 128:
            break
    _, slot_of, col_of = best
    core_of = slot_of // TILES
    tile_of = slot_of % TILES
    loc_of = tile_of * 128 + col_of
    pid_of = core_of * NP + loc_of
    return src, dst, dinv, core_of, tile_of, col_of, loc_of, pid_of


def _build_streams(src, dst, core_of, tile_of, col_of, loc_of):
    """Per-core edge streams with uniform (tile, half) chunk counts."""
    N, TILES, BTILES = _CFG["N"], _CFG["TILES"], _CFG["BTILES"]
    TL, SZ_LO, SZ_HI = _CFG["TL"], _CFG["SZ_LO"], _CFG["SZ_HI"]
    a_src, a_dst = src, dst

    e_core = core_of[a_dst]
    e_tile = tile_of[a_dst]
    e_dcol = col_of[a_dst]
    e_hi = (tile_of[a_src] >= TL).astype(np.int64)
    e_spid = np.where(
        e_hi == 0,
        core_of[a_src] * SZ_LO + loc_of[a_src],
        core_of[a_src] * SZ_HI + (loc_of[a_src] - SZ_LO))

    key = (e_core * TILES + e_tile) * 2 + e_hi
    cnt = np.bincount(key, minlength=NCORE * TILES * 2).reshape(NCORE, TILES, 2)
    K_lo = max(1, int(np.ceil(cnt[:, :, 0].max() / 128)))
    K_hi = max(1, int(np.ceil(cnt[:, :, 1].max() / 128)))

    order = np.lexsort((e_spid, e_hi, e_tile, e_core))
    e_core, e_tile, e_dcol, e_spid, e_hi = (
        e_core[order], e_tile[order], e_dcol[order], e_spid[order], e_hi[order])

    batches = []
    t = 0
    while t < TILES:
        batches.append(list(range(t, min(t + BTILES, TILES))))
        t += BTILES

    flat = cnt.reshape(-1)
    csum = np.concatenate([[0], np.cumsum(flat)])
    starts = csum[:-1].reshape(NCORE, TILES, 2)

    per_core = []
    for c in range(NCORE):
        idx_segs = []
        dst_cols = []
        for bt in batches:
            for h in range(2):
                K = K_lo if h == 0 else K_hi
                seg_idx = np.zeros((len(bt) * K * 128,), np.int64)
                seg_dst = np.full((len(bt) * K * 128,), -1.0, np.float32)
                for j, t in enumerate(bt):
                    s0 = starts[c, t, h]
                    n = cnt[c, t, h]
                    sl = slice(j * K * 128, j * K * 128 + n)
                    seg_idx[sl] = e_spid[s0 : s0 + n]
                    seg_dst[sl] = e_dcol[s0 : s0 + n]
                w = seg_idx.astype(np.int16).reshape(-1, 16).T.copy()
                idx_segs.append(np.tile(w, (8, 1)))
                dst_cols.append(
                    seg_dst.reshape(-1, 128).T.copy().astype(np.float16))
        per_core.append((np.concatenate(idx_segs, axis=1),
                         np.concatenate(dst_cols, axis=1)))

    meta = dict(K_lo=K_lo, K_hi=K_hi, batches=batches)
    return per_core, meta


# ----------------------------------------------------------------------------
# device kernel
# ----------------------------------------------------------------------------

def _build_nc(meta, FT, CT):
    import concourse.bacc as bacc
    import concourse.mybir as mybir
    from concourse import tile

    N, TILES, NP = _CFG["N"], _CFG["TILES"], _CFG["NP"]
    NPAD, LASTV = _CFG["NPAD"], _CFG["LASTV"]
    TL, SZ_LO, SZ_HI = _CFG["TL"], _CFG["SZ_LO"], _CFG["SZ_HI"]
    HALF_LO, HALF_HI = _CFG["HALF_LO"], _CFG["HALF_HI"]
    F16, F32, I16 = mybir.dt.float16, mybir.dt.float32, mybir.dt.int16
    K_lo, K_hi, batches = meta["K_lo"], meta["K_hi"], meta["batches"]

    NOCC = bool(int(os.environ.get("GCN_NOCC", "0")))
    NOGATHER = bool(int(os.environ.get("GCN_NOGATHER", "0")))
    NOAGG = bool(int(os.environ.get("GCN_NOAGG", "0")))
    NOBN = bool(int(os.environ.get("GCN_NOBN", "0")))
    nc = bacc.Bacc(num_swdge_queues=2)
    t_xT = nc.declare_dram_parameter("xT", [DIN, NP], F32, isOutput=False)
    t_idx = nc.declare_dram_parameter("idx", [128, FT], I16, isOutput=False)
    t_dst = nc.declare_dram_parameter("dst", [128, CT], F16, isOutput=False)
    t_iota = nc.declare_dram_parameter("iota", [128, 128], F16, isOutput=False)
    t_idn = nc.declare_dram_parameter("idn", [64, 64], F16, isOutput=False)
    t_dvr = nc.declare_dram_parameter("dvr", [64, NP], F16, isOutput=False)
    t_dvl = nc.declare_dram_parameter("dvl", [128, TILES], F32, isOutput=False)
    t_W1 = nc.declare_dram_parameter("W1", [DIN, DOUT], F32, isOutput=False)
    t_W2 = nc.declare_dram_parameter("W2", [DOUT, DOUT], F16, isOutput=False)
    t_W3 = nc.declare_dram_parameter("W3", [DOUT, DIN], F16, isOutput=False)
    t_p12 = nc.declare_dram_parameter("p12", [64, 4], F32, isOutput=False)
    t_p3 = nc.declare_dram_parameter("p3", [128, 4], F32, isOutput=False)
    o_out = nc.declare_dram_parameter("outT", [2, 128, NP], F32, isOutput=True)

    h_locA = [nc.dram_tensor(f"h_locA{l}", [SZ_LO, 2 * DOUT], F16)
              for l in range(3)]
    h_locB = [nc.dram_tensor(f"h_locB{l}", [SZ_HI, 2 * DOUT], F16)
              for l in range(3)]
    h_tabA = [nc.dram_tensor(f"h_tabA{l}", [HALF_LO, 2 * DOUT], F16,
                             addr_space="Shared") for l in range(3)]
    h_tabB = [nc.dram_tensor(f"h_tabB{l}", [HALF_HI, 2 * DOUT], F16,
                             addr_space="Shared") for l in range(3)]
    strows = [64, 64, 128]
    st_in = [nc.dram_tensor(f"st_in{l}", [strows[l], 4], F32) for l in range(3)]
    st_out = [nc.dram_tensor(f"st_out{l}", [strows[l], 4], F32,
                             addr_space="Shared") for l in range(3)]

    CB = len(batches[0]) * (K_lo + K_hi)
    RG = [list(range(NCORE))]

    with tile.TileContext(nc) as tc:
        with (
            tc.tile_pool(name="const", bufs=1) as pc,
            tc.tile_pool(name="work", bufs=2) as pw,
            tc.tile_pool(name="gat", bufs=1) as pg,
            tc.tile_pool(name="psA", bufs=2, space="PSUM") as psA,
            tc.tile_pool(name="psC", bufs=2, space="PSUM") as psC,
            tc.tile_pool(name="psB", bufs=2, space="PSUM") as psB,
        ):
            # ---- persistent loads ----
            s_idx = pc.tile([128, FT], I16)
            nc.sync.dma_start(out=s_idx[:], in_=t_idx[:])
            s_dst = pc.tile([128, CT], F16)
            nc.sync.dma_start(out=s_dst[:], in_=t_dst[:])
            s_iota = pc.tile([128, 128], F16)
            nc.sync.dma_start(out=s_iota[:], in_=t_iota[:])
            s_idn = pc.tile([64, 64], F16)
            nc.sync.dma_start(out=s_idn[:], in_=t_idn[:])
            s_dvr = pc.tile([64, NP], F16)
            nc.sync.dma_start(out=s_dvr[:], in_=t_dvr[:])
            s_dvl = pc.tile([128, TILES], F32)
            nc.sync.dma_start(out=s_dvl[:], in_=t_dvl[:])
            s_W1 = pc.tile([128, 2, DOUT], F32)
            nc.sync.dma_start(
                out=s_W1[:], in_=t_W1[:].rearrange("(k p) f -> p k f", p=128))
            s_W2 = pc.tile([DOUT, DOUT], F16)
            nc.sync.dma_start(out=s_W2[:], in_=t_W2[:])
            s_W3 = pc.tile([DOUT, DIN], F16)
            nc.sync.dma_start(out=s_W3[:], in_=t_W3[:])
            s_p12 = pc.tile([64, 4], F32)
            nc.sync.dma_start(out=s_p12[:], in_=t_p12[:])
            s_p3 = pc.tile([128, 4], F32)
            nc.sync.dma_start(out=s_p3[:], in_=t_p3[:])

            s_aT = pc.tile([64, NP], F16)
            s_tabT = pc.tile([64, NP], F16)
            s_z3 = pc.tile([128, 2, NP], F32, tag="zz")
            s_z12 = pc.tile([64, NP], F32, tag="zz")
            s_hst = pc.tile([128, TILES, 2 * DOUT], F16)
            s_stat = pc.tile([128, 8], F32)
            nc.vector.memset(s_hst[:], 0.0)
            s_vec = pc.tile([128, 8], F32)

            def build_table(l):
                for t in range(TILES):
                    tr = slice(t * 128, (t + 1) * 128)
                    if l == 0:
                        ph = psB.tile([128, DOUT], F32, tag="ph")
                        phT = psB.tile([64, 128], F32, tag="phT")
                        xt = pw.tile([128, 2, 128], F32, tag="xt")
                        nc.sync.dma_start(
                            out=xt[:],
                            in_=t_xT[:, tr].rearrange("(k p) n -> p k n", p=128))
                        for k in range(2):
                            nc.tensor.matmul(
                                ph[:], xt[:, k, :], s_W1[:, k, :],
                                start=(k == 0), stop=(k == 1))
                        for k in range(2):
                            nc.tensor.matmul(
                                phT[:], s_W1[:, k, :], xt[:, k, :],
                                start=(k == 0), stop=(k == 1))
                        nc.vector.tensor_tensor(
                            s_tabT[:, tr], phT[:], s_dvr[:, tr],
                            mybir.AluOpType.mult)
                    elif l == 1:
                        ph = psB.tile([128, DOUT], F32, tag="ph")
                        phT = psB.tile([64, 128], F32, tag="phT")
                        nc.tensor.matmul(
                            ph[:], s_aT[:, tr], s_W2[:], start=True, stop=True)
                        nc.tensor.matmul(
                            phT[:], s_W2[:], s_aT[:, tr], start=True, stop=True)
                        nc.vector.tensor_tensor(
                            s_tabT[:, tr], phT[:], s_dvr[:, tr],
                            mybir.AluOpType.mult)
                    else:
                        ph = psB.tile([128, DOUT], F16, tag="ph")
                        nc.tensor.transpose(ph[:], s_aT[:, tr], s_idn[:])
                        nc.vector.tensor_tensor(
                            s_tabT[:, tr], s_aT[:, tr], s_dvr[:, tr],
                            mybir.AluOpType.mult)
                    nc.vector.tensor_scalar(
                        s_hst[:, t, 0:DOUT], ph[:], s_dvl[:, t:t + 1], None,
                        mybir.AluOpType.mult)
                    if t == TL - 1:
                        nc.sync.dma_start(
                            out=h_locA[l][:].rearrange(
                                "(t p) f -> p t f", p=128),
                            in_=s_hst[:, 0:TL, :])
                        if NOCC:
                            nc.sync.dma_start(out=h_tabA[l][0:SZ_LO, :],
                                              in_=h_locA[l][:])
                        else:
                            nc.gpsimd.collective_compute(
                                "AllGather", mybir.AluOpType.bypass,
                                replica_groups=RG,
                                ins=[h_locA[l][:].opt()],
                                outs=[h_tabA[l][:].opt()])
                nc.sync.dma_start(
                    out=h_locB[l][:].rearrange(
                        "(t p) f -> p t f", p=128),
                    in_=s_hst[:, TL:TILES, :])
                if NOCC:
                    nc.sync.dma_start(out=h_tabB[l][0:SZ_HI, :],
                                      in_=h_locB[l][:])
                else:
                    nc.gpsimd.collective_compute(
                        "AllGather", mybir.AluOpType.bypass, replica_groups=RG,
                        ins=[h_locB[l][:].opt()],
                        outs=[h_tabB[l][:].opt()])

            def aggregate(l):
                if NOAGG:
                    if l < 2:
                        nc.vector.memset(s_z12[:], 0.0)
                    else:
                        nc.vector.memset(s_z3[:], 0.0)
                    return
                cb0 = 0
                f0 = 0
                for bt in batches:
                    nb = len(bt)
                    n_lo, n_hi = nb * K_lo * 128, nb * K_hi * 128
                    CBb = nb * (K_lo + K_hi)
                    m16 = pw.tile([128, CB, 2 * DOUT], F16, tag="m16")
                    if NOGATHER:
                        nc.vector.memset(m16[:], 0.0)
                    else:
                        nc.gpsimd.dma_gather(
                            m16[:, 0:nb * K_lo, :], h_tabA[l][:, :],
                            s_idx[:, f0:f0 + n_lo // 16], n_lo, n_lo, 2 * DOUT,
                            single_packet=False, queue_num=0)
                        nc.gpsimd.dma_gather(
                            m16[:, nb * K_lo:CBb, :], h_tabB[l][:, :],
                            s_idx[:, f0 + n_lo // 16:f0 + (n_lo + n_hi) // 16],
                            n_hi, n_hi, 2 * DOUT,
                            single_packet=False, queue_num=1)
                    f0 += (n_lo + n_hi) // 16
                    sS = pw.tile([128, CB, 128], F16, tag="sS")
                    nc.vector.tensor_tensor(
                        sS[:, 0:CBb, :],
                        s_iota[:].unsqueeze(1).broadcast_to([128, CBb, 128]),
                        s_dst[:, cb0:cb0 + CBb].unsqueeze(2).broadcast_to(
                            [128, CBb, 128]),
                        mybir.AluOpType.is_equal)
                    for j, t in enumerate(bt):
                        pa = psA.tile([64, 128], F32, tag="pa")
                        ck = (list(range(j * K_lo, (j + 1) * K_lo))
                              + list(range(nb * K_lo + j * K_hi,
                                           nb * K_lo + (j + 1) * K_hi)))
                        for i, c in enumerate(ck):
                            nc.tensor.matmul(
                                pa[:], m16[:, c, 0:DOUT], sS[:, c, :],
                                start=(i == 0), stop=(i == len(ck) - 1))
                        tr = slice(t * 128, (t + 1) * 128)
                        if l < 2:
                            nc.vector.tensor_tensor(
                                s_z12[:, tr], pa[:], s_tabT[:, tr],
                                mybir.AluOpType.add)
                            nc.vector.tensor_tensor(
                                s_z12[:, tr], s_z12[:, tr], s_dvr[:, tr],
                                mybir.AluOpType.mult)
                        else:
                            ag = pw.tile([64, 128], F16, tag="ag")
                            nc.vector.tensor_tensor(
                                ag[:], pa[:], s_tabT[:, tr], mybir.AluOpType.add)
                            nc.vector.tensor_tensor(
                                ag[:], ag[:], s_dvr[:, tr], mybir.AluOpType.mult)
                            for hf in range(2):
                                p3p = psC.tile([128, 128], F32, tag="p3p")
                                nc.tensor.matmul(
                                    p3p[:], s_W3[:, hf * 128:(hf + 1) * 128],
                                    ag[:], start=True, stop=True)
                                nc.vector.tensor_copy(s_z3[:, hf, tr], p3p[:])
                    cb0 += CBb

            def bn_stats(l):
                if NOBN:
                    nc.vector.memset(s_stat[:], 0.0)
                    nc.vector.memset(s_vec[:], 1.0)
                    return
                nrows = strows[l]
                CH = 7 * 128
                NCH = (NP + CH - 1) // CH
                if l < 2:
                    sq = pw.tile([64, CH], F32, tag="sq")
                    pt = pw.tile([64, NCH], F32, tag="pt")
                    nc.vector.reduce_sum(
                        s_stat[0:64, 0:1], s_z12[:, :], axis=mybir.AxisListType.X)
                    for i in range(NCH):
                        w = min(CH, NP - i * CH)
                        nc.vector.tensor_tensor(
                            sq[:, 0:w], s_z12[:, i * CH:i * CH + w],
                            s_z12[:, i * CH:i * CH + w], mybir.AluOpType.mult)
                        nc.vector.reduce_sum(
                            pt[:, i:i + 1], sq[:, 0:w], axis=mybir.AxisListType.X)
                    nc.vector.reduce_sum(
                        s_stat[0:64, 1:2], pt[:], axis=mybir.AxisListType.X)
                    nc.vector.memset(s_stat[0:64, 2:4], 0.0)
                else:
                    sq = pw.tile([128, CH], F32, tag="sq3")
                    pt = pw.tile([128, NCH], F32, tag="pt3")
                    for hf in range(2):
                        nc.vector.reduce_sum(
                            s_stat[:, 2 * hf:2 * hf + 1], s_z3[:, hf, :],
                            axis=mybir.AxisListType.X)
                        for i in range(NCH):
                            w = min(CH, NP - i * CH)
                            nc.vector.tensor_tensor(
                                sq[:, 0:w], s_z3[:, hf, i * CH:i * CH + w],
                                s_z3[:, hf, i * CH:i * CH + w],
                                mybir.AluOpType.mult)
                            nc.vector.reduce_sum(
                                pt[:, i:i + 1], sq[:, 0:w],
                                axis=mybir.AxisListType.X)
                        nc.vector.reduce_sum(
                            s_stat[:, 2 * hf + 1:2 * hf + 2], pt[:],
                            axis=mybir.AxisListType.X)
                nc.sync.dma_start(out=st_in[l][:], in_=s_stat[0:nrows, 0:4])
                if NOCC:
                    nc.sync.dma_start(out=st_out[l][:], in_=st_in[l][:])
                else:
                    nc.gpsimd.collective_compute(
                        "AllReduce", mybir.AluOpType.add, replica_groups=RG,
                        ins=[st_in[l][:].opt()], outs=[st_out[l][:].opt()])
                nc.sync.dma_start(out=s_stat[0:nrows, 4:8], in_=st_out[l][:])
                invN = 1.0 / float(N)
                npair = 1 if l < 2 else 2
                for p in range(npair):
                    r = slice(0, nrows)
                    su = s_stat[r, 4 + 2 * p:5 + 2 * p]
                    s2 = s_stat[r, 5 + 2 * p:6 + 2 * p]
                    m = s_vec[r, 4:5]
                    nc.vector.tensor_scalar(m, su, invN, None, mybir.AluOpType.mult)
                    ex2 = s_vec[r, 5:6]
                    nc.vector.tensor_scalar(s2, s2, invN, None, mybir.AluOpType.mult)
                    nc.vector.tensor_tensor(ex2, m, m, mybir.AluOpType.mult)
                    nc.vector.tensor_tensor(ex2, s2, ex2, mybir.AluOpType.subtract)
                    sd = s_vec[r, 6:7]
                    nc.vector.tensor_scalar(ex2, ex2, float(EPS), None,
                                            mybir.AluOpType.add)
                    nc.scalar.activation(sd, ex2, mybir.ActivationFunctionType.Sqrt)
                    inv = s_vec[r, 7:8]
                    nc.vector.reciprocal(inv, sd)
                    if l < 2:
                        g = s_p12[:, 2 * l:2 * l + 1]
                        be = s_p12[:, 2 * l + 1:2 * l + 2]
                    else:
                        g = s_p3[:, p:p + 1]
                        be = s_p3[:, 2 + p:3 + p]
                    sc = s_vec[r, 2 * p:2 * p + 1]
                    sh = s_vec[r, 2 * p + 1:2 * p + 2]
                    nc.vector.tensor_tensor(sc, g, inv, mybir.AluOpType.mult)
                    nc.vector.tensor_tensor(sh, m, sc, mybir.AluOpType.mult)
                    nc.vector.tensor_tensor(sh, be, sh, mybir.AluOpType.subtract)

            stage = os.environ.get("GCN_STAGE", "full")
            if stage == "gdump":
                bt = batches[0]
                nb = len(bt)
                n_lo, n_hi = nb * K_lo * 128, nb * K_hi * 128
                CBb = nb * (K_lo + K_hi)
                o_dbg = nc.declare_dram_parameter(
                    "dbg", [128, CB, 2 * DOUT], F16, isOutput=True)
                build_table(0)
                m16 = pw.tile([128, CB, 2 * DOUT], F16, tag="m16")
                nc.vector.memset(m16[:], 0.0)
                nc.gpsimd.dma_gather(
                    m16[:, 0:nb * K_lo, :], h_tabA[0][:, :],
                    s_idx[:, 0:n_lo // 16], n_lo, n_lo, 2 * DOUT,
                    single_packet=False, queue_num=0)
                nc.gpsimd.dma_gather(
                    m16[:, nb * K_lo:CBb, :], h_tabB[0][:, :],
                    s_idx[:, n_lo // 16:(n_lo + n_hi) // 16], n_hi, n_hi,
                    2 * DOUT, single_packet=False, queue_num=1)
                nc.sync.dma_start(out=o_dbg[:], in_=m16[:])
                nc.gpsimd.dma_start(out=o_out[0][:, 0:TILES * DOUT],
                                    in_=s_hst[:, :, 0:DOUT])
            elif stage == "tabdump":
                o_dbg = nc.declare_dram_parameter(
                    "dbg", [NCORE * 992, 2 * DOUT], F16, isOutput=True)
                build_table(0)
                half = os.environ.get("GCN_DUMPHALF", "A")
                for c8 in range(NCORE):
                    if half == "A":
                        nc.sync.dma_start(
                            out=o_dbg[c8 * 992:(c8 + 1) * 992, :],
                            in_=h_tabA[0][(c8 + 1) * SZ_LO - 992:(c8 + 1) * SZ_LO, :])
                    else:
                        nc.sync.dma_start(
                            out=o_dbg[c8 * 992:(c8 + 1) * 992, :],
                            in_=h_tabB[0][(c8 + 1) * SZ_HI - 992:(c8 + 1) * SZ_HI, :])
                nc.gpsimd.dma_start(out=o_out[0][:, 0:TILES * DOUT],
                                    in_=s_hst[:, :, 0:DOUT])
            elif stage == "tableng":
                # table build without the collective (timing control)
                for t in range(TILES):
                    tr = slice(t * 128, (t + 1) * 128)
                    ph = psB.tile([128, DOUT], F32, tag="ph")
                    xt = pw.tile([128, 2, 128], F32, tag="xt")
                    nc.sync.dma_start(
                        out=xt[:],
                        in_=t_xT[:, tr].rearrange("(k p) n -> p k n", p=128))
                    for k in range(2):
                        nc.tensor.matmul(ph[:], xt[:, k, :], s_W1[:, k, :],
                                         start=(k == 0), stop=(k == 1))
                    nc.vector.tensor_scalar(
                        s_hst[:, t, 0:DOUT], ph[:], s_dvl[:, t:t + 1], None,
                        mybir.AluOpType.mult)
                nc.sync.dma_start(
                    out=h_locA[0][:].rearrange("(t p) f -> p t f", p=128),
                    in_=s_hst[:, 0:TL, :])
                nc.gpsimd.dma_start(out=o_out[0][:, 0:TILES * DOUT],
                                    in_=s_hst[:, :, 0:DOUT])
            elif stage == "gonly":
                # table + collective + gathers only (no cast/S/matmul)
                build_table(0)
                f0 = 0
                cb0 = 0
                for bt in batches:
                    nb = len(bt)
                    n_lo, n_hi = nb * K_lo * 128, nb * K_hi * 128
                    CBb = nb * (K_lo + K_hi)
                    m32 = pg.tile([128, CB, 2 * DOUT], F16, tag="m32")
                    nc.gpsimd.dma_gather(
                        m32[:, 0:nb * K_lo, :], h_tabA[0][:, :],
                        s_idx[:, f0:f0 + n_lo // 16], n_lo, n_lo, 2 * DOUT,
                        single_packet=False)
                    f0 += n_lo // 16
                    nc.gpsimd.dma_gather(
                        m32[:, nb * K_lo:CBb, :], h_tabB[0][:, :],
                        s_idx[:, f0:f0 + n_hi // 16], n_hi, n_hi, 2 * DOUT,
                        single_packet=False)
                    f0 += n_hi // 16
                    cb0 += CBb
                nc.gpsimd.dma_start(out=o_out[0][:, 0:TILES * DOUT],
                                    in_=s_hst[:, :, 0:DOUT])
            elif stage == "table":
                build_table(0)
                nc.gpsimd.dma_start(out=o_out[0][:, 0:TILES * DOUT],
                                    in_=s_hst[:, :, 0:DOUT])
                nc.vector.memset(s_z3[:, 1, 0:128], 0.0)
                nc.sync.dma_start(out=o_out[1][:, 0:128], in_=s_z3[:, 1, 0:128])
            elif stage == "agg":
                build_table(0)
                aggregate(0)
                nc.sync.dma_start(out=o_out[0][0:64, :], in_=s_z12[:, :])
                nc.sync.dma_start(out=o_out[1][0:64, :], in_=s_z12[:, :])
            if stage == "full":
                for l in range(2):
                    build_table(l)
                    aggregate(l)
                    bn_stats(l)
                    nc.scalar.activation(
                        s_aT[:], s_z12[:, :],
                        mybir.ActivationFunctionType.Relu,
                        bias=s_vec[0:64, 1:2], scale=s_vec[0:64, 0:1])
                build_table(2)
                aggregate(2)
                bn_stats(2)
                RCH = 7 * 128
                for hf in range(2):
                    nc.scalar.activation(
                        s_z3[:, hf, :], s_z3[:, hf, :],
                        mybir.ActivationFunctionType.Identity,
                        bias=s_vec[:, 2 * hf + 1:2 * hf + 2],
                        scale=s_vec[:, 2 * hf:2 * hf + 1])
                    for i in range((NP + RCH - 1) // RCH):
                        w = min(RCH, NP - i * RCH)
                        rs = slice(i * RCH, i * RCH + w)
                        xt = pw.tile([128, RCH], F32, tag="xr")
                        nc.sync.dma_start(
                            out=xt[:, 0:w], in_=t_xT[hf * 128:(hf + 1) * 128, rs])
                        nc.vector.tensor_tensor(
                            s_z3[:, hf, rs], s_z3[:, hf, rs], xt[:, 0:w],
                            mybir.AluOpType.add)
                        nc.vector.tensor_scalar(
                            s_z3[:, hf, rs], s_z3[:, hf, rs], 0.0, None,
                            mybir.AluOpType.max)
                        nc.sync.dma_start(out=o_out[hf][:, rs],
                                          in_=s_z3[:, hf, rs])

    nc.finalize()
    return nc


# ----------------------------------------------------------------------------
# entry point
# ----------------------------------------------------------------------------

def _prepare(x, ei, W1, g1, be1, W2, g2, be2, W3, g3, be3):
    N, NP, TILES = _CFG["N"], _CFG["NP"], _CFG["TILES"]
    x = np.asarray(x, np.float32)
    ei = np.asarray(ei, np.int32)
    src, dst, dinv, core_of, tile_of, col_of, loc_of, pid_of = _prep_graph(ei)
    per_core, meta = _build_streams(src, dst, core_of, tile_of, col_of, loc_of)

    iota = np.tile(np.arange(128, dtype=np.float16)[None, :], (128, 1))
    idn = np.eye(64, dtype=np.float16)
    p12 = np.stack([np.asarray(g1), np.asarray(be1),
                    np.asarray(g2), np.asarray(be2)], axis=1).astype(np.float32)
    g3c = np.asarray(g3, np.float32).reshape(2, 128).T
    be3c = np.asarray(be3, np.float32).reshape(2, 128).T
    p3 = np.concatenate([g3c, be3c], axis=1).astype(np.float32)

    in_maps = []
    for c in range(NCORE):
        nodes_c = np.nonzero(core_of == c)[0]
        lidx = loc_of[nodes_c]
        xT = np.zeros((DIN, NP), np.float32)
        xT[:, lidx] = x[nodes_c].T
        dvr = np.zeros((NP,), np.float32)
        dvr[lidx] = dinv[nodes_c]
        dvl = dvr.reshape(TILES, 128).T.copy()
        idx_all, dst_all = per_core[c]
        in_maps.append({
            "xT": xT, "idx": np.ascontiguousarray(idx_all),
            "dst": np.ascontiguousarray(dst_all), "iota": iota, "idn": idn,
            "dvr": np.tile(dvr[None, :], (64, 1)).astype(np.float16),
            "dvl": np.ascontiguousarray(dvl),
            "W1": np.asarray(W1, np.float32),
            "W2": np.asarray(W2, np.float32).astype(np.float16),
            "W3": np.asarray(W3, np.float32).astype(np.float16),
            "p12": p12, "p3": p3,
        })
    return in_maps, meta, core_of, loc_of


def kernel(x, ei, batch, W1, b1, g1, be1, W2, b2, g2, be2, W3, b3, g3, be3):
    global LAST_EXEC_NS
    from concourse.bass_utils import run_bass_kernel_spmd

    N, NP = _CFG["N"], _CFG["NP"]
    in_maps, meta, core_of, loc_of = _prepare(
        x, ei, W1, g1, be1, W2, g2, be2, W3, g3, be3)
    nc = _build_nc(meta, in_maps[0]["idx"].shape[1], in_maps[0]["dst"].shape[1])

    trace = bool(int(os.environ.get("GCN_TRACE", "0")))
    res = run_bass_kernel_spmd(nc, in_maps, list(range(NCORE)), trace=trace)
    if res.exec_time_ns is not None:
        LAST_EXEC_NS = res.exec_time_ns

    out = np.empty((N, DIN), np.float32)
    for c in range(NCORE):
        nodes_c = np.nonzero(core_of == c)[0]
        arr = res.results[c]["outT"].reshape(DIN, NP)
        out[nodes_c] = arr[:, loc_of[nodes_c]].T
    return out



# revision 21
# speedup vs baseline: 1.0569x; 1.0569x over previous
"""Trainium2 Bass kernel for a 3-layer GCN bottleneck block (50k nodes, 800k edges).

Strategy (8 NeuronCores, dst-node sharding):
- Host: relabel nodes into 8 cores x TILES tiles x 128 slots, balancing per-tile
  in-degree. Edges sorted by (dst tile, src half, src id); each (tile, half)
  group padded to a uniform chunk count so one SPMD program serves all cores.
  Self-loops become plain edges.
- All three convs aggregate in 64-dim space (conv3 rewritten as (A~ @ a2) @ W3).
- Per layer: compute local h-shard = a @ W with dinv[src] folded in, AllGather
  the [NPAD, 64] fp32 table, dma_gather 256B rows per edge (src-sorted for HBM
  locality), segment-sum via one-hot matmul: aggT[64f,128d] += M16.T @ S where
  S = is_equal(iota, dstloc) built in one DVE pass per batch (pads get dstloc=-1
  so their S column is zero). dinv[dst] applied during PSUM eviction.
- BatchNorm: per-core feature-major partial sums, tiny AllReduce, ACT-fused
  scale/bias/relu. Final layer: W3 matmul to 256-dim, BN3, residual, relu,
  output transposed; host untransposes/unpermutes.
"""

import os
import numpy as np

DIN = 256
DOUT = 64
EPS = 1e-5
NCORE = 8

# default (real-problem) geometry; test_sim.py overrides via configure()
_CFG = {}


def configure(N, E, NLOC, BTILES):
    TILES = (NLOC + 127) // 128
    NP = TILES * 128
    TL = (TILES + 1) // 2                     # lo tiles 0..TL-1
    SZ_LO, SZ_HI = TL * 128, NP - TL * 128
    _CFG.update(
        N=N, E=E, NLOC=NLOC, TILES=TILES, NP=NP,
        NPAD=NCORE * NP,
        TL=TL, SZ_LO=SZ_LO, SZ_HI=SZ_HI,
        HALF_LO=NCORE * SZ_LO, HALF_HI=NCORE * SZ_HI,
        LASTV=NLOC - 128 * (TILES - 1),       # valid rows in last tile
        BTILES=BTILES,
    )


configure(N=50000, E=800000, NLOC=6250, BTILES=3)

LAST_EXEC_NS = None    # set by kernel() when GCN_TRACE=1


# ----------------------------------------------------------------------------
# host-side graph prep
# ----------------------------------------------------------------------------

def _prep_graph(ei):
    import heapq
    N, TILES, NP = _CFG["N"], _CFG["TILES"], _CFG["NP"]
    LASTV = _CFG["LASTV"]
    src, dst = ei[0].astype(np.int64), ei[1].astype(np.int64)
    deg = np.bincount(dst, minlength=N).astype(np.float32) + 1.0
    dinv = (1.0 / np.sqrt(deg)).astype(np.float32)
    indeg = np.bincount(dst, minlength=N)

    # Balance BOTH lo and hi in-edge loads per (core,tile) slot: the padded
    # chunk counts K_lo/K_hi are set by the max over slots, so minimize the
    # larger of the two running loads. lo/hi of an edge depends on the SOURCE
    # node's final core, unknown during assignment — approximate with a
    # first-pass assignment by total degree, then rebalance on realized lo/hi.
    nslot = NCORE * TILES
    cap = np.full(nslot, 128, np.int64)
    cap[TILES - 1 :: TILES] = LASTV

    def greedy(key_lo, key_hi):
        order = np.argsort(-(key_lo + key_hi), kind="stable")
        load_lo = np.zeros(nslot, np.float64)
        load_hi = np.zeros(nslot, np.float64)
        fill = np.zeros(nslot, np.int64)
        slot_of = np.empty(N, np.int64)
        col_of = np.empty(N, np.int64)
        heap = [(0.0, s) for s in range(nslot)]
        heapq.heapify(heap)
        for n in order:
            while True:
                l, s = heapq.heappop(heap)
                if fill[s] < cap[s] and l == max(load_lo[s], load_hi[s]):
                    break
                if fill[s] < cap[s]:
                    heapq.heappush(heap, (max(load_lo[s], load_hi[s]), s))
            slot_of[n] = s
            col_of[n] = fill[s]
            fill[s] += 1
            load_lo[s] += key_lo[n]
            load_hi[s] += key_hi[n]
            if fill[s] < cap[s]:
                heapq.heappush(heap, (max(load_lo[s], load_hi[s]), s))
        return slot_of, col_of

    # pass 1: split unknown -> assume half/half
    half = indeg.astype(np.float64) / 2.0
    slot_of, col_of = greedy(half, half)
    # refine: realized lo/hi per dst node under previous pass's tiles
    best = None
    for _ in range(6):
        tile1 = slot_of % TILES
        src_hi = tile1[src] >= _CFG["TL"]
        lo_cnt = np.bincount(dst[~src_hi], minlength=N).astype(np.float64)
        hi_cnt = np.bincount(dst[src_hi], minlength=N).astype(np.float64)
        slot_of, col_of = greedy(lo_cnt, hi_cnt)
        # realized max per (slot, half) under THIS assignment
        tile2 = slot_of % TILES
        s_hi = tile2[src] >= _CFG["TL"]
        e_slot = slot_of[dst]
        sl_self_lo = (np.arange(N) % 1 == 0)  # self loop: src tile == own tile
        lo_l = np.bincount(e_slot[~s_hi], minlength=nslot).astype(np.int64)
        hi_l = np.bincount(e_slot[s_hi], minlength=nslot).astype(np.int64)
        mx = max(lo_l.max(), hi_l.max())
        if best is None or mx < best[0]:
            best = (mx, slot_of.copy(), col_of.copy())
        if mx <= 9 * 128:
            break
    _, slot_of, col_of = best
    core_of = slot_of // TILES
    tile_of = slot_of % TILES
    loc_of = tile_of * 128 + col_of
    pid_of = core_of * NP + loc_of
    return src, dst, dinv, core_of, tile_of, col_of, loc_of, pid_of


def _build_streams(src, dst, core_of, tile_of, col_of, loc_of):
    """Per-core edge streams with uniform (tile, half) chunk counts."""
    N, TILES, BTILES = _CFG["N"], _CFG["TILES"], _CFG["BTILES"]
    TL, SZ_LO, SZ_HI = _CFG["TL"], _CFG["SZ_LO"], _CFG["SZ_HI"]
    a_src, a_dst = src, dst

    e_core = core_of[a_dst]
    e_tile = tile_of[a_dst]
    e_dcol = col_of[a_dst]
    e_hi = (tile_of[a_src] >= TL).astype(np.int64)
    e_spid = np.where(
        e_hi == 0,
        core_of[a_src] * SZ_LO + loc_of[a_src],
        core_of[a_src] * SZ_HI + (loc_of[a_src] - SZ_LO))

    key = (e_core * TILES + e_tile) * 2 + e_hi
    cnt = np.bincount(key, minlength=NCORE * TILES * 2).reshape(NCORE, TILES, 2)
    K_lo = max(1, int(np.ceil(cnt[:, :, 0].max() / 128)))
    K_hi = max(1, int(np.ceil(cnt[:, :, 1].max() / 128)))

    order = np.lexsort((e_spid, e_hi, e_tile, e_core))
    e_core, e_tile, e_dcol, e_spid, e_hi = (
        e_core[order], e_tile[order], e_dcol[order], e_spid[order], e_hi[order])

    batches = []
    t = 0
    while t < TILES:
        batches.append(list(range(t, min(t + BTILES, TILES))))
        t += BTILES

    flat = cnt.reshape(-1)
    csum = np.concatenate([[0], np.cumsum(flat)])
    starts = csum[:-1].reshape(NCORE, TILES, 2)

    per_core = []
    for c in range(NCORE):
        idx_segs = []
        dst_cols = []
        for bt in batches:
            for h in range(2):
                K = K_lo if h == 0 else K_hi
                seg_idx = np.zeros((len(bt) * K * 128,), np.int64)
                seg_dst = np.full((len(bt) * K * 128,), -1.0, np.float32)
                for j, t in enumerate(bt):
                    s0 = starts[c, t, h]
                    n = cnt[c, t, h]
                    sl = slice(j * K * 128, j * K * 128 + n)
                    seg_idx[sl] = e_spid[s0 : s0 + n]
                    seg_dst[sl] = e_dcol[s0 : s0 + n]
                w = seg_idx.astype(np.int16).reshape(-1, 16).T.copy()
                idx_segs.append(np.tile(w, (8, 1)))
                dst_cols.append(
                    seg_dst.reshape(-1, 128).T.copy().astype(np.float16))
        per_core.append((np.concatenate(idx_segs, axis=1),
                         np.concatenate(dst_cols, axis=1)))

    meta = dict(K_lo=K_lo, K_hi=K_hi, batches=batches)
    return per_core, meta


# ----------------------------------------------------------------------------
# device kernel
# ----------------------------------------------------------------------------

def _build_nc(meta, FT, CT):
    import concourse.bacc as bacc
    import concourse.mybir as mybir
    from concourse import tile

    N, TILES, NP = _CFG["N"], _CFG["TILES"], _CFG["NP"]
    NPAD, LASTV = _CFG["NPAD"], _CFG["LASTV"]
    TL, SZ_LO, SZ_HI = _CFG["TL"], _CFG["SZ_LO"], _CFG["SZ_HI"]
    HALF_LO, HALF_HI = _CFG["HALF_LO"], _CFG["HALF_HI"]
    F16, F32, I16 = mybir.dt.float16, mybir.dt.float32, mybir.dt.int16
    K_lo, K_hi, batches = meta["K_lo"], meta["K_hi"], meta["batches"]

    NOCC = bool(int(os.environ.get("GCN_NOCC", "0")))
    NOGATHER = bool(int(os.environ.get("GCN_NOGATHER", "0")))
    NOAGG = bool(int(os.environ.get("GCN_NOAGG", "0")))
    NOBN = bool(int(os.environ.get("GCN_NOBN", "0")))
    nc = bacc.Bacc(num_swdge_queues=2)
    t_xT = nc.declare_dram_parameter("xT", [DIN, NP], F32, isOutput=False)
    t_idx = nc.declare_dram_parameter("idx", [128, FT], I16, isOutput=False)
    t_dst = nc.declare_dram_parameter("dst", [128, CT], F16, isOutput=False)
    t_iota = nc.declare_dram_parameter("iota", [128, 128], F16, isOutput=False)
    t_idn = nc.declare_dram_parameter("idn", [64, 64], F16, isOutput=False)
    t_dvr = nc.declare_dram_parameter("dvr", [64, NP], F16, isOutput=False)
    t_dvl = nc.declare_dram_parameter("dvl", [128, TILES], F32, isOutput=False)
    t_W1 = nc.declare_dram_parameter("W1", [DIN, DOUT], F32, isOutput=False)
    t_W2 = nc.declare_dram_parameter("W2", [DOUT, DOUT], F16, isOutput=False)
    t_W3 = nc.declare_dram_parameter("W3", [DOUT, DIN], F16, isOutput=False)
    t_p12 = nc.declare_dram_parameter("p12", [64, 4], F32, isOutput=False)
    t_p3 = nc.declare_dram_parameter("p3", [128, 4], F32, isOutput=False)
    o_out = nc.declare_dram_parameter("outT", [2, 128, NP], F32, isOutput=True)

    h_locA = [nc.dram_tensor(f"h_locA{l}", [SZ_LO, 2 * DOUT], F16)
              for l in range(3)]
    h_locB = [nc.dram_tensor(f"h_locB{l}", [SZ_HI, 2 * DOUT], F16)
              for l in range(3)]
    h_tabA = [nc.dram_tensor(f"h_tabA{l}", [HALF_LO, 2 * DOUT], F16,
                             addr_space="Shared") for l in range(3)]
    h_tabB = [nc.dram_tensor(f"h_tabB{l}", [HALF_HI, 2 * DOUT], F16,
                             addr_space="Shared") for l in range(3)]
    strows = [64, 64, 128]
    st_in = [nc.dram_tensor(f"st_in{l}", [strows[l], 4], F32) for l in range(3)]
    st_out = [nc.dram_tensor(f"st_out{l}", [strows[l], 4], F32,
                             addr_space="Shared") for l in range(3)]

    CB = len(batches[0]) * (K_lo + K_hi)
    RG = [list(range(NCORE))]

    with tile.TileContext(nc) as tc:
        with (
            tc.tile_pool(name="const", bufs=1) as pc,
            tc.tile_pool(name="work", bufs=3) as pw,
            tc.tile_pool(name="gat", bufs=1) as pg,
            tc.tile_pool(name="psA", bufs=2, space="PSUM") as psA,
            tc.tile_pool(name="psC", bufs=2, space="PSUM") as psC,
            tc.tile_pool(name="psB", bufs=2, space="PSUM") as psB,
        ):
            # ---- persistent loads ----
            s_idx = pc.tile([128, FT], I16)
            nc.sync.dma_start(out=s_idx[:], in_=t_idx[:])
            s_dst = pc.tile([128, CT], F16)
            nc.sync.dma_start(out=s_dst[:], in_=t_dst[:])
            s_iota = pc.tile([128, 128], F16)
            nc.sync.dma_start(out=s_iota[:], in_=t_iota[:])
            s_idn = pc.tile([64, 64], F16)
            nc.sync.dma_start(out=s_idn[:], in_=t_idn[:])
            s_dvr = pc.tile([64, NP], F16)
            nc.sync.dma_start(out=s_dvr[:], in_=t_dvr[:])
            s_dvl = pc.tile([128, TILES], F32)
            nc.sync.dma_start(out=s_dvl[:], in_=t_dvl[:])
            s_W1 = pc.tile([128, 2, DOUT], F32)
            nc.sync.dma_start(
                out=s_W1[:], in_=t_W1[:].rearrange("(k p) f -> p k f", p=128))
            s_W2 = pc.tile([DOUT, DOUT], F16)
            nc.sync.dma_start(out=s_W2[:], in_=t_W2[:])
            s_W3 = pc.tile([DOUT, DIN], F16)
            nc.sync.dma_start(out=s_W3[:], in_=t_W3[:])
            s_p12 = pc.tile([64, 4], F32)
            nc.sync.dma_start(out=s_p12[:], in_=t_p12[:])
            s_p3 = pc.tile([128, 4], F32)
            nc.sync.dma_start(out=s_p3[:], in_=t_p3[:])

            s_aT = pc.tile([64, NP], F16)
            s_tabT = pc.tile([64, NP], F16)
            s_z3 = pc.tile([128, 2, NP], F16, tag="zz")
            s_z12 = pc.tile([64, NP], F32, tag="zz")
            s_hst = pc.tile([128, TILES, 2 * DOUT], F16)
            s_stat = pc.tile([128, 8], F32)
            nc.vector.memset(s_hst[:], 0.0)
            s_vec = pc.tile([128, 8], F32)

            def build_table(l):
                for t in range(TILES):
                    tr = slice(t * 128, (t + 1) * 128)
                    if l == 0:
                        ph = psB.tile([128, DOUT], F32, tag="ph")
                        phT = psB.tile([64, 128], F32, tag="phT")
                        xt = pw.tile([128, 2, 128], F32, tag="xt")
                        nc.sync.dma_start(
                            out=xt[:],
                            in_=t_xT[:, tr].rearrange("(k p) n -> p k n", p=128))
                        for k in range(2):
                            nc.tensor.matmul(
                                ph[:], xt[:, k, :], s_W1[:, k, :],
                                start=(k == 0), stop=(k == 1))
                        for k in range(2):
                            nc.tensor.matmul(
                                phT[:], s_W1[:, k, :], xt[:, k, :],
                                start=(k == 0), stop=(k == 1))
                        nc.vector.tensor_tensor(
                            s_tabT[:, tr], phT[:], s_dvr[:, tr],
                            mybir.AluOpType.mult)
                    elif l == 1:
                        ph = psB.tile([128, DOUT], F32, tag="ph")
                        phT = psB.tile([64, 128], F32, tag="phT")
                        nc.tensor.matmul(
                            ph[:], s_aT[:, tr], s_W2[:], start=True, stop=True)
                        nc.tensor.matmul(
                            phT[:], s_W2[:], s_aT[:, tr], start=True, stop=True)
                        nc.vector.tensor_tensor(
                            s_tabT[:, tr], phT[:], s_dvr[:, tr],
                            mybir.AluOpType.mult)
                    else:
                        ph = psB.tile([128, DOUT], F16, tag="ph")
                        nc.tensor.transpose(ph[:], s_aT[:, tr], s_idn[:])
                        nc.vector.tensor_tensor(
                            s_tabT[:, tr], s_aT[:, tr], s_dvr[:, tr],
                            mybir.AluOpType.mult)
                    nc.vector.tensor_scalar(
                        s_hst[:, t, 0:DOUT], ph[:], s_dvl[:, t:t + 1], None,
                        mybir.AluOpType.mult)
                    if t == TL - 1:
                        nc.sync.dma_start(
                            out=h_locA[l][:].rearrange(
                                "(t p) f -> p t f", p=128),
                            in_=s_hst[:, 0:TL, :])
                        if NOCC:
                            nc.sync.dma_start(out=h_tabA[l][0:SZ_LO, :],
                                              in_=h_locA[l][:])
                        else:
                            nc.gpsimd.collective_compute(
                                "AllGather", mybir.AluOpType.bypass,
                                replica_groups=RG,
                                ins=[h_locA[l][:].opt()],
                                outs=[h_tabA[l][:].opt()])
                nc.sync.dma_start(
                    out=h_locB[l][:].rearrange(
                        "(t p) f -> p t f", p=128),
                    in_=s_hst[:, TL:TILES, :])
                if NOCC:
                    nc.sync.dma_start(out=h_tabB[l][0:SZ_HI, :],
                                      in_=h_locB[l][:])
                else:
                    nc.gpsimd.collective_compute(
                        "AllGather", mybir.AluOpType.bypass, replica_groups=RG,
                        ins=[h_locB[l][:].opt()],
                        outs=[h_tabB[l][:].opt()])

            def aggregate(l):
                if NOAGG:
                    if l < 2:
                        nc.vector.memset(s_z12[:], 0.0)
                    else:
                        nc.vector.memset(s_z3[:], 0.0)
                    return
                cb0 = 0
                f0 = 0
                for bt in batches:
                    nb = len(bt)
                    n_lo, n_hi = nb * K_lo * 128, nb * K_hi * 128
                    CBb = nb * (K_lo + K_hi)
                    m16 = pw.tile([128, CB, 2 * DOUT], F16, tag="m16")
                    if NOGATHER:
                        nc.vector.memset(m16[:], 0.0)
                    else:
                        nc.gpsimd.dma_gather(
                            m16[:, 0:nb * K_lo, :], h_tabA[l][:, :],
                            s_idx[:, f0:f0 + n_lo // 16], n_lo, n_lo, 2 * DOUT,
                            single_packet=False, queue_num=0)
                        nc.gpsimd.dma_gather(
                            m16[:, nb * K_lo:CBb, :], h_tabB[l][:, :],
                            s_idx[:, f0 + n_lo // 16:f0 + (n_lo + n_hi) // 16],
                            n_hi, n_hi, 2 * DOUT,
                            single_packet=False, queue_num=1)
                    f0 += (n_lo + n_hi) // 16
                    sS = pw.tile([128, CB, 128], F16, tag="sS")
                    nc.vector.tensor_tensor(
                        sS[:, 0:CBb, :],
                        s_iota[:].unsqueeze(1).broadcast_to([128, CBb, 128]),
                        s_dst[:, cb0:cb0 + CBb].unsqueeze(2).broadcast_to(
                            [128, CBb, 128]),
                        mybir.AluOpType.is_equal)
                    for j, t in enumerate(bt):
                        pa = psA.tile([64, 128], F32, tag="pa")
                        ck = (list(range(j * K_lo, (j + 1) * K_lo))
                              + list(range(nb * K_lo + j * K_hi,
                                           nb * K_lo + (j + 1) * K_hi)))
                        for i, c in enumerate(ck):
                            nc.tensor.matmul(
                                pa[:], m16[:, c, 0:DOUT], sS[:, c, :],
                                start=(i == 0), stop=(i == len(ck) - 1))
                        tr = slice(t * 128, (t + 1) * 128)
                        if l < 2:
                            nc.vector.tensor_tensor(
                                s_z12[:, tr], pa[:], s_tabT[:, tr],
                                mybir.AluOpType.add)
                            nc.vector.tensor_tensor(
                                s_z12[:, tr], s_z12[:, tr], s_dvr[:, tr],
                                mybir.AluOpType.mult)
                        else:
                            ag = pw.tile([64, 128], F16, tag="ag")
                            nc.vector.tensor_tensor(
                                ag[:], pa[:], s_tabT[:, tr], mybir.AluOpType.add)
                            nc.vector.tensor_tensor(
                                ag[:], ag[:], s_dvr[:, tr], mybir.AluOpType.mult)
                            for hf in range(2):
                                p3p = psC.tile([128, 128], F32, tag="p3p")
                                nc.tensor.matmul(
                                    p3p[:], s_W3[:, hf * 128:(hf + 1) * 128],
                                    ag[:], start=True, stop=True)
                                nc.vector.tensor_copy(s_z3[:, hf, tr], p3p[:])
                    cb0 += CBb

            def bn_stats(l):
                if NOBN:
                    nc.vector.memset(s_stat[:], 0.0)
                    nc.vector.memset(s_vec[:], 1.0)
                    return
                nrows = strows[l]
                CH = 7 * 128
                NCH = (NP + CH - 1) // CH
                if l < 2:
                    sq = pw.tile([64, CH], F32, tag="sq")
                    pt = pw.tile([64, NCH], F32, tag="pt")
                    nc.vector.reduce_sum(
                        s_stat[0:64, 0:1], s_z12[:, :], axis=mybir.AxisListType.X)
                    for i in range(NCH):
                        w = min(CH, NP - i * CH)
                        nc.vector.tensor_tensor(
                            sq[:, 0:w], s_z12[:, i * CH:i * CH + w],
                            s_z12[:, i * CH:i * CH + w], mybir.AluOpType.mult)
                        nc.vector.reduce_sum(
                            pt[:, i:i + 1], sq[:, 0:w], axis=mybir.AxisListType.X)
                    nc.vector.reduce_sum(
                        s_stat[0:64, 1:2], pt[:], axis=mybir.AxisListType.X)
                    nc.vector.memset(s_stat[0:64, 2:4], 0.0)
                else:
                    sq = pw.tile([128, CH], F32, tag="sq3")
                    pt = pw.tile([128, NCH], F32, tag="pt3")
                    for hf in range(2):
                        nc.vector.reduce_sum(
                            s_stat[:, 2 * hf:2 * hf + 1], s_z3[:, hf, :],
                            axis=mybir.AxisListType.X)
                        for i in range(NCH):
                            w = min(CH, NP - i * CH)
                            nc.vector.tensor_tensor(
                                sq[:, 0:w], s_z3[:, hf, i * CH:i * CH + w],
                                s_z3[:, hf, i * CH:i * CH + w],
                                mybir.AluOpType.mult)
                            nc.vector.reduce_sum(
                                pt[:, i:i + 1], sq[:, 0:w],
                                axis=mybir.AxisListType.X)
                        nc.vector.reduce_sum(
                            s_stat[:, 2 * hf + 1:2 * hf + 2], pt[:],
                            axis=mybir.AxisListType.X)
                nc.sync.dma_start(out=st_in[l][:], in_=s_stat[0:nrows, 0:4])
                if NOCC:
                    nc.sync.dma_start(out=st_out[l][:], in_=st_in[l][:])
                else:
                    nc.gpsimd.collective_compute(
                        "AllReduce", mybir.AluOpType.add, replica_groups=RG,
                        ins=[st_in[l][:].opt()], outs=[st_out[l][:].opt()])
                nc.sync.dma_start(out=s_stat[0:nrows, 4:8], in_=st_out[l][:])
                invN = 1.0 / float(N)
                npair = 1 if l < 2 else 2
                for p in range(npair):
                    r = slice(0, nrows)
                    su = s_stat[r, 4 + 2 * p:5 + 2 * p]
                    s2 = s_stat[r, 5 + 2 * p:6 + 2 * p]
                    m = s_vec[r, 4:5]
                    nc.vector.tensor_scalar(m, su, invN, None, mybir.AluOpType.mult)
                    ex2 = s_vec[r, 5:6]
                    nc.vector.tensor_scalar(s2, s2, invN, None, mybir.AluOpType.mult)
                    nc.vector.tensor_tensor(ex2, m, m, mybir.AluOpType.mult)
                    nc.vector.tensor_tensor(ex2, s2, ex2, mybir.AluOpType.subtract)
                    sd = s_vec[r, 6:7]
                    nc.vector.tensor_scalar(ex2, ex2, float(EPS), None,
                                            mybir.AluOpType.add)
                    nc.scalar.activation(sd, ex2, mybir.ActivationFunctionType.Sqrt)
                    inv = s_vec[r, 7:8]
                    nc.vector.reciprocal(inv, sd)
                    if l < 2:
                        g = s_p12[:, 2 * l:2 * l + 1]
                        be = s_p12[:, 2 * l + 1:2 * l + 2]
                    else:
                        g = s_p3[:, p:p + 1]
                        be = s_p3[:, 2 + p:3 + p]
                    sc = s_vec[r, 2 * p:2 * p + 1]
                    sh = s_vec[r, 2 * p + 1:2 * p + 2]
                    nc.vector.tensor_tensor(sc, g, inv, mybir.AluOpType.mult)
                    nc.vector.tensor_tensor(sh, m, sc, mybir.AluOpType.mult)
                    nc.vector.tensor_tensor(sh, be, sh, mybir.AluOpType.subtract)

            stage = os.environ.get("GCN_STAGE", "full")
            if stage == "gdump":
                bt = batches[0]
                nb = len(bt)
                n_lo, n_hi = nb * K_lo * 128, nb * K_hi * 128
                CBb = nb * (K_lo + K_hi)
                o_dbg = nc.declare_dram_parameter(
                    "dbg", [128, CB, 2 * DOUT], F16, isOutput=True)
                build_table(0)
                m16 = pw.tile([128, CB, 2 * DOUT], F16, tag="m16")
                nc.vector.memset(m16[:], 0.0)
                nc.gpsimd.dma_gather(
                    m16[:, 0:nb * K_lo, :], h_tabA[0][:, :],
                    s_idx[:, 0:n_lo // 16], n_lo, n_lo, 2 * DOUT,
                    single_packet=False, queue_num=0)
                nc.gpsimd.dma_gather(
                    m16[:, nb * K_lo:CBb, :], h_tabB[0][:, :],
                    s_idx[:, n_lo // 16:(n_lo + n_hi) // 16], n_hi, n_hi,
                    2 * DOUT, single_packet=False, queue_num=1)
                nc.sync.dma_start(out=o_dbg[:], in_=m16[:])
                nc.gpsimd.dma_start(out=o_out[0][:, 0:TILES * DOUT],
                                    in_=s_hst[:, :, 0:DOUT])
            elif stage == "tabdump":
                o_dbg = nc.declare_dram_parameter(
                    "dbg", [NCORE * 992, 2 * DOUT], F16, isOutput=True)
                build_table(0)
                half = os.environ.get("GCN_DUMPHALF", "A")
                for c8 in range(NCORE):
                    if half == "A":
                        nc.sync.dma_start(
                            out=o_dbg[c8 * 992:(c8 + 1) * 992, :],
                            in_=h_tabA[0][(c8 + 1) * SZ_LO - 992:(c8 + 1) * SZ_LO, :])
                    else:
                        nc.sync.dma_start(
                            out=o_dbg[c8 * 992:(c8 + 1) * 992, :],
                            in_=h_tabB[0][(c8 + 1) * SZ_HI - 992:(c8 + 1) * SZ_HI, :])
                nc.gpsimd.dma_start(out=o_out[0][:, 0:TILES * DOUT],
                                    in_=s_hst[:, :, 0:DOUT])
            elif stage == "tableng":
                # table build without the collective (timing control)
                for t in range(TILES):
                    tr = slice(t * 128, (t + 1) * 128)
                    ph = psB.tile([128, DOUT], F32, tag="ph")
                    xt = pw.tile([128, 2, 128], F32, tag="xt")
                    nc.sync.dma_start(
                        out=xt[:],
                        in_=t_xT[:, tr].rearrange("(k p) n -> p k n", p=128))
                    for k in range(2):
                        nc.tensor.matmul(ph[:], xt[:, k, :], s_W1[:, k, :],
                                         start=(k == 0), stop=(k == 1))
                    nc.vector.tensor_scalar(
                        s_hst[:, t, 0:DOUT], ph[:], s_dvl[:, t:t + 1], None,
                        mybir.AluOpType.mult)
                nc.sync.dma_start(
                    out=h_locA[0][:].rearrange("(t p) f -> p t f", p=128),
                    in_=s_hst[:, 0:TL, :])
                nc.gpsimd.dma_start(out=o_out[0][:, 0:TILES * DOUT],
                                    in_=s_hst[:, :, 0:DOUT])
            elif stage == "gonly":
                # table + collective + gathers only (no cast/S/matmul)
                build_table(0)
                f0 = 0
                cb0 = 0
                for bt in batches:
                    nb = len(bt)
                    n_lo, n_hi = nb * K_lo * 128, nb * K_hi * 128
                    CBb = nb * (K_lo + K_hi)
                    m32 = pg.tile([128, CB, 2 * DOUT], F16, tag="m32")
                    nc.gpsimd.dma_gather(
                        m32[:, 0:nb * K_lo, :], h_tabA[0][:, :],
                        s_idx[:, f0:f0 + n_lo // 16], n_lo, n_lo, 2 * DOUT,
                        single_packet=False)
                    f0 += n_lo // 16
                    nc.gpsimd.dma_gather(
                        m32[:, nb * K_lo:CBb, :], h_tabB[0][:, :],
                        s_idx[:, f0:f0 + n_hi // 16], n_hi, n_hi, 2 * DOUT,
                        single_packet=False)
                    f0 += n_hi // 16
                    cb0 += CBb
                nc.gpsimd.dma_start(out=o_out[0][:, 0:TILES * DOUT],
                                    in_=s_hst[:, :, 0:DOUT])
            elif stage == "table":
                build_table(0)
                nc.gpsimd.dma_start(out=o_out[0][:, 0:TILES * DOUT],
                                    in_=s_hst[:, :, 0:DOUT])
                nc.vector.memset(s_z3[:, 1, 0:128], 0.0)
                nc.gpsimd.dma_start(out=o_out[1][:, 0:128], in_=s_z3[:, 1, 0:128])
            elif stage == "agg":
                build_table(0)
                aggregate(0)
                nc.sync.dma_start(out=o_out[0][0:64, :], in_=s_z12[:, :])
                nc.sync.dma_start(out=o_out[1][0:64, :], in_=s_z12[:, :])
            if stage == "full":
                for l in range(2):
                    build_table(l)
                    aggregate(l)
                    bn_stats(l)
                    nc.scalar.activation(
                        s_aT[:], s_z12[:, :],
                        mybir.ActivationFunctionType.Relu,
                        bias=s_vec[0:64, 1:2], scale=s_vec[0:64, 0:1])
                build_table(2)
                aggregate(2)
                bn_stats(2)
                RCH = 7 * 128
                for hf in range(2):
                    nc.scalar.activation(
                        s_z3[:, hf, :], s_z3[:, hf, :],
                        mybir.ActivationFunctionType.Identity,
                        bias=s_vec[:, 2 * hf + 1:2 * hf + 2],
                        scale=s_vec[:, 2 * hf:2 * hf + 1])
                    for i in range((NP + RCH - 1) // RCH):
                        w = min(RCH, NP - i * RCH)
                        rs = slice(i * RCH, i * RCH + w)
                        xt = pw.tile([128, RCH], F32, tag="xr")
                        nc.sync.dma_start(
                            out=xt[:, 0:w], in_=t_xT[hf * 128:(hf + 1) * 128, rs])
                        nc.vector.tensor_tensor(
                            s_z3[:, hf, rs], s_z3[:, hf, rs], xt[:, 0:w],
                            mybir.AluOpType.add)
                        nc.vector.tensor_scalar(
                            s_z3[:, hf, rs], s_z3[:, hf, rs], 0.0, None,
                            mybir.AluOpType.max)
                        nc.gpsimd.dma_start(out=o_out[hf][:, rs],
                                            in_=s_z3[:, hf, rs])

    nc.finalize()
    return nc


# ----------------------------------------------------------------------------
# entry point
# ----------------------------------------------------------------------------

def _prepare(x, ei, W1, g1, be1, W2, g2, be2, W3, g3, be3):
    N, NP, TILES = _CFG["N"], _CFG["NP"], _CFG["TILES"]
    x = np.asarray(x, np.float32)
    ei = np.asarray(ei, np.int32)
    src, dst, dinv, core_of, tile_of, col_of, loc_of, pid_of = _prep_graph(ei)
    per_core, meta = _build_streams(src, dst, core_of, tile_of, col_of, loc_of)

    iota = np.tile(np.arange(128, dtype=np.float16)[None, :], (128, 1))
    idn = np.eye(64, dtype=np.float16)
    p12 = np.stack([np.asarray(g1), np.asarray(be1),
                    np.asarray(g2), np.asarray(be2)], axis=1).astype(np.float32)
    g3c = np.asarray(g3, np.float32).reshape(2, 128).T
    be3c = np.asarray(be3, np.float32).reshape(2, 128).T
    p3 = np.concatenate([g3c, be3c], axis=1).astype(np.float32)

    in_maps = []
    for c in range(NCORE):
        nodes_c = np.nonzero(core_of == c)[0]
        lidx = loc_of[nodes_c]
        xT = np.zeros((DIN, NP), np.float32)
        xT[:, lidx] = x[nodes_c].T
        dvr = np.zeros((NP,), np.float32)
        dvr[lidx] = dinv[nodes_c]
        dvl = dvr.reshape(TILES, 128).T.copy()
        idx_all, dst_all = per_core[c]
        in_maps.append({
            "xT": xT, "idx": np.ascontiguousarray(idx_all),
            "dst": np.ascontiguousarray(dst_all), "iota": iota, "idn": idn,
            "dvr": np.tile(dvr[None, :], (64, 1)).astype(np.float16),
            "dvl": np.ascontiguousarray(dvl),
            "W1": np.asarray(W1, np.float32),
            "W2": np.asarray(W2, np.float32).astype(np.float16),
            "W3": np.asarray(W3, np.float32).astype(np.float16),
            "p12": p12, "p3": p3,
        })
    return in_maps, meta, core_of, loc_of


def kernel(x, ei, batch, W1, b1, g1, be1, W2, b2, g2, be2, W3, b3, g3, be3):
    global LAST_EXEC_NS
    from concourse.bass_utils import run_bass_kernel_spmd

    N, NP = _CFG["N"], _CFG["NP"]
    in_maps, meta, core_of, loc_of = _prepare(
        x, ei, W1, g1, be1, W2, g2, be2, W3, g3, be3)
    nc = _build_nc(meta, in_maps[0]["idx"].shape[1], in_maps[0]["dst"].shape[1])

    trace = bool(int(os.environ.get("GCN_TRACE", "0")))
    res = run_bass_kernel_spmd(nc, in_maps, list(range(NCORE)), trace=trace)
    if res.exec_time_ns is not None:
        LAST_EXEC_NS = res.exec_time_ns

    out = np.empty((N, DIN), np.float32)
    for c in range(NCORE):
        nodes_c = np.nonzero(core_of == c)[0]
        arr = res.results[c]["outT"].reshape(DIN, NP)
        out[nodes_c] = arr[:, loc_of[nodes_c]].T
    return out



# revision 24
# speedup vs baseline: 2.1705x; 2.0535x over previous
"""Trainium2 Bass kernel for a 3-layer GCN bottleneck block (50k nodes, 800k edges).

Strategy (8 NeuronCores, dst-node sharding):
- Host: relabel nodes into 8 cores x TILES tiles x 128 slots, balancing per-tile
  in-degree. Edges sorted by (dst tile, src half, src id); each (tile, half)
  group padded to a uniform chunk count so one SPMD program serves all cores.
  Self-loops become plain edges.
- All three convs aggregate in 64-dim space (conv3 rewritten as (A~ @ a2) @ W3).
- Per layer: compute local h-shard = a @ W with dinv[src] folded in, AllGather
  the [NPAD, 64] fp32 table, dma_gather 256B rows per edge (src-sorted for HBM
  locality), segment-sum via one-hot matmul: aggT[64f,128d] += M16.T @ S where
  S = is_equal(iota, dstloc) built in one DVE pass per batch (pads get dstloc=-1
  so their S column is zero). dinv[dst] applied during PSUM eviction.
- BatchNorm: per-core feature-major partial sums, tiny AllReduce, ACT-fused
  scale/bias/relu. Final layer: W3 matmul to 256-dim, BN3, residual, relu,
  output transposed; host untransposes/unpermutes.
"""

import os
import numpy as np

DIN = 256
DOUT = 64
EPS = 1e-5
NCORE = 8

# default (real-problem) geometry; test_sim.py overrides via configure()
_CFG = {}


def configure(N, E, NLOC, BTILES):
    TILES = (NLOC + 127) // 128
    NP = TILES * 128
    TL = (TILES + 1) // 2                     # lo tiles 0..TL-1
    SZ_LO, SZ_HI = TL * 128, NP - TL * 128
    _CFG.update(
        N=N, E=E, NLOC=NLOC, TILES=TILES, NP=NP,
        NPAD=NCORE * NP,
        TL=TL, SZ_LO=SZ_LO, SZ_HI=SZ_HI,
        HALF_LO=NCORE * SZ_LO, HALF_HI=NCORE * SZ_HI,
        LASTV=NLOC - 128 * (TILES - 1),       # valid rows in last tile
        BTILES=BTILES,
    )


configure(N=50000, E=800000, NLOC=6250, BTILES=4)

LAST_EXEC_NS = None    # set by kernel() when GCN_TRACE=1


# ----------------------------------------------------------------------------
# host-side graph prep
# ----------------------------------------------------------------------------

def _prep_graph(ei):
    import heapq
    N, TILES, NP = _CFG["N"], _CFG["TILES"], _CFG["NP"]
    LASTV = _CFG["LASTV"]
    src, dst = ei[0].astype(np.int64), ei[1].astype(np.int64)
    deg = np.bincount(dst, minlength=N).astype(np.float32) + 1.0
    dinv = (1.0 / np.sqrt(deg)).astype(np.float32)
    indeg = np.bincount(dst, minlength=N)

    # Balance BOTH lo and hi in-edge loads per (core,tile) slot: the padded
    # chunk counts K_lo/K_hi are set by the max over slots, so minimize the
    # larger of the two running loads. lo/hi of an edge depends on the SOURCE
    # node's final core, unknown during assignment — approximate with a
    # first-pass assignment by total degree, then rebalance on realized lo/hi.
    nslot = NCORE * TILES
    cap = np.full(nslot, 128, np.int64)
    cap[TILES - 1 :: TILES] = LASTV

    def greedy(key_lo, key_hi):
        order = np.argsort(-(key_lo + key_hi), kind="stable")
        load_lo = np.zeros(nslot, np.float64)
        load_hi = np.zeros(nslot, np.float64)
        fill = np.zeros(nslot, np.int64)
        slot_of = np.empty(N, np.int64)
        col_of = np.empty(N, np.int64)
        heap = [(0.0, s) for s in range(nslot)]
        heapq.heapify(heap)
        for n in order:
            while True:
                l, s = heapq.heappop(heap)
                if fill[s] < cap[s] and l == max(load_lo[s], load_hi[s]):
                    break
                if fill[s] < cap[s]:
                    heapq.heappush(heap, (max(load_lo[s], load_hi[s]), s))
            slot_of[n] = s
            col_of[n] = fill[s]
            fill[s] += 1
            load_lo[s] += key_lo[n]
            load_hi[s] += key_hi[n]
            if fill[s] < cap[s]:
                heapq.heappush(heap, (max(load_lo[s], load_hi[s]), s))
        return slot_of, col_of

    # pass 1: split unknown -> assume half/half
    half = indeg.astype(np.float64) / 2.0
    slot_of, col_of = greedy(half, half)
    # refine: realized lo/hi per dst node under previous pass's tiles
    best = None
    for _ in range(6):
        tile1 = slot_of % TILES
        src_hi = tile1[src] >= _CFG["TL"]
        lo_cnt = np.bincount(dst[~src_hi], minlength=N).astype(np.float64)
        hi_cnt = np.bincount(dst[src_hi], minlength=N).astype(np.float64)
        slot_of, col_of = greedy(lo_cnt, hi_cnt)
        # realized max per (slot, half) under THIS assignment
        tile2 = slot_of % TILES
        s_hi = tile2[src] >= _CFG["TL"]
        e_slot = slot_of[dst]
        sl_self_lo = (np.arange(N) % 1 == 0)  # self loop: src tile == own tile
        lo_l = np.bincount(e_slot[~s_hi], minlength=nslot).astype(np.int64)
        hi_l = np.bincount(e_slot[s_hi], minlength=nslot).astype(np.int64)
        mx = max(lo_l.max(), hi_l.max())
        if best is None or mx < best[0]:
            best = (mx, slot_of.copy(), col_of.copy())
        if mx <= 9 * 128:
            break
    _, slot_of, col_of = best
    core_of = slot_of // TILES
    tile_of = slot_of % TILES
    loc_of = tile_of * 128 + col_of
    pid_of = core_of * NP + loc_of
    return src, dst, dinv, core_of, tile_of, col_of, loc_of, pid_of


def _build_streams(src, dst, core_of, tile_of, col_of, loc_of):
    """Per-core edge streams with uniform (tile, half) chunk counts."""
    N, TILES, BTILES = _CFG["N"], _CFG["TILES"], _CFG["BTILES"]
    TL, SZ_LO, SZ_HI = _CFG["TL"], _CFG["SZ_LO"], _CFG["SZ_HI"]
    a_src, a_dst = src, dst

    e_core = core_of[a_dst]
    e_tile = tile_of[a_dst]
    e_dcol = col_of[a_dst]
    e_hi = (tile_of[a_src] >= TL).astype(np.int64)
    e_spid = np.where(
        e_hi == 0,
        core_of[a_src] * SZ_LO + loc_of[a_src],
        core_of[a_src] * SZ_HI + (loc_of[a_src] - SZ_LO))

    key = (e_core * TILES + e_tile) * 2 + e_hi
    cnt = np.bincount(key, minlength=NCORE * TILES * 2).reshape(NCORE, TILES, 2)
    K_lo = max(1, int(np.ceil(cnt[:, :, 0].max() / 128)))
    K_hi = max(1, int(np.ceil(cnt[:, :, 1].max() / 128)))

    order = np.lexsort((e_spid, e_hi, e_tile, e_core))
    e_core, e_tile, e_dcol, e_spid, e_hi = (
        e_core[order], e_tile[order], e_dcol[order], e_spid[order], e_hi[order])

    batches = []
    t = 0
    while t < TILES:
        batches.append(list(range(t, min(t + BTILES, TILES))))
        t += BTILES

    flat = cnt.reshape(-1)
    csum = np.concatenate([[0], np.cumsum(flat)])
    starts = csum[:-1].reshape(NCORE, TILES, 2)

    per_core = []
    for c in range(NCORE):
        idx_segs = []
        dst_cols = []
        for bt in batches:
            for h in range(2):
                K = K_lo if h == 0 else K_hi
                seg_idx = np.zeros((len(bt) * K * 128,), np.int64)
                seg_dst = np.full((len(bt) * K * 128,), -1.0, np.float32)
                for j, t in enumerate(bt):
                    s0 = starts[c, t, h]
                    n = cnt[c, t, h]
                    sl = slice(j * K * 128, j * K * 128 + n)
                    seg_idx[sl] = e_spid[s0 : s0 + n]
                    seg_dst[sl] = e_dcol[s0 : s0 + n]
                w = seg_idx.astype(np.int16).reshape(-1, 16).T.copy()
                idx_segs.append(np.tile(w, (8, 1)))
                dst_cols.append(
                    seg_dst.reshape(-1, 128).T.copy().astype(np.float16))
        per_core.append((np.concatenate(idx_segs, axis=1),
                         np.concatenate(dst_cols, axis=1)))

    meta = dict(K_lo=K_lo, K_hi=K_hi, batches=batches)
    return per_core, meta


# ----------------------------------------------------------------------------
# device kernel
# ----------------------------------------------------------------------------

def _build_nc(meta, FT, CT):
    import concourse.bacc as bacc
    import concourse.mybir as mybir
    from concourse import tile

    N, TILES, NP = _CFG["N"], _CFG["TILES"], _CFG["NP"]
    NPAD, LASTV = _CFG["NPAD"], _CFG["LASTV"]
    TL, SZ_LO, SZ_HI = _CFG["TL"], _CFG["SZ_LO"], _CFG["SZ_HI"]
    HALF_LO, HALF_HI = _CFG["HALF_LO"], _CFG["HALF_HI"]
    F16, F32, I16 = mybir.dt.float16, mybir.dt.float32, mybir.dt.int16
    K_lo, K_hi, batches = meta["K_lo"], meta["K_hi"], meta["batches"]

    NOCC = bool(int(os.environ.get("GCN_NOCC", "0")))
    NOGATHER = bool(int(os.environ.get("GCN_NOGATHER", "0")))
    NOAGG = bool(int(os.environ.get("GCN_NOAGG", "0")))
    NOBN = bool(int(os.environ.get("GCN_NOBN", "0")))
    nc = bacc.Bacc(num_swdge_queues=2)
    t_xT = nc.declare_dram_parameter("xT", [DIN, NP], F32, isOutput=False)
    t_idx = nc.declare_dram_parameter("idx", [128, FT], I16, isOutput=False)
    t_dst = nc.declare_dram_parameter("dst", [128, CT], F16, isOutput=False)
    t_iota = nc.declare_dram_parameter("iota", [128, 128], F16, isOutput=False)
    t_idn = nc.declare_dram_parameter("idn", [64, 64], F16, isOutput=False)
    t_dvr = nc.declare_dram_parameter("dvr", [64, NP], F16, isOutput=False)
    t_dvl = nc.declare_dram_parameter("dvl", [128, TILES], F32, isOutput=False)
    t_W1 = nc.declare_dram_parameter("W1", [DIN, DOUT], F32, isOutput=False)
    t_W2 = nc.declare_dram_parameter("W2", [DOUT, DOUT], F16, isOutput=False)
    t_W3 = nc.declare_dram_parameter("W3", [DOUT, DIN], F16, isOutput=False)
    t_p12 = nc.declare_dram_parameter("p12", [64, 4], F32, isOutput=False)
    t_p3 = nc.declare_dram_parameter("p3", [128, 4], F32, isOutput=False)
    o_out = nc.declare_dram_parameter("outT", [2, 128, NP], F32, isOutput=True)

    h_locA = [nc.dram_tensor(f"h_locA{l}", [SZ_LO, 2 * DOUT], F16)
              for l in range(3)]
    h_locB = [nc.dram_tensor(f"h_locB{l}", [SZ_HI, 2 * DOUT], F16)
              for l in range(3)]
    h_tabA = [nc.dram_tensor(f"h_tabA{l}", [HALF_LO, 2 * DOUT], F16,
                             addr_space="Shared") for l in range(3)]
    h_tabB = [nc.dram_tensor(f"h_tabB{l}", [HALF_HI, 2 * DOUT], F16,
                             addr_space="Shared") for l in range(3)]
    strows = [64, 64, 128]
    st_in = [nc.dram_tensor(f"st_in{l}", [strows[l], 4], F32) for l in range(3)]
    st_out = [nc.dram_tensor(f"st_out{l}", [strows[l], 4], F32,
                             addr_space="Shared") for l in range(3)]

    CB = len(batches[0]) * (K_lo + K_hi)
    RG = [list(range(NCORE))]

    with tile.TileContext(nc) as tc:
        with (
            tc.tile_pool(name="const", bufs=1) as pc,
            tc.tile_pool(name="work", bufs=2) as pw,
            tc.tile_pool(name="gat", bufs=1) as pg,
            tc.tile_pool(name="psA", bufs=2, space="PSUM") as psA,
            tc.tile_pool(name="psC", bufs=2, space="PSUM") as psC,
            tc.tile_pool(name="psB", bufs=2, space="PSUM") as psB,
        ):
            # ---- persistent loads ----
            s_idx = pc.tile([128, FT], I16)
            nc.sync.dma_start(out=s_idx[:], in_=t_idx[:])
            s_dst = pc.tile([128, CT], F16)
            nc.sync.dma_start(out=s_dst[:], in_=t_dst[:])
            s_iota = pc.tile([128, 128], F16)
            nc.sync.dma_start(out=s_iota[:], in_=t_iota[:])
            s_idn = pc.tile([64, 64], F16)
            nc.sync.dma_start(out=s_idn[:], in_=t_idn[:])
            s_dvr = pc.tile([64, NP], F16)
            nc.sync.dma_start(out=s_dvr[:], in_=t_dvr[:])
            s_dvl = pc.tile([128, TILES], F32)
            nc.sync.dma_start(out=s_dvl[:], in_=t_dvl[:])
            s_W1 = pc.tile([128, 2, DOUT], F32)
            nc.sync.dma_start(
                out=s_W1[:], in_=t_W1[:].rearrange("(k p) f -> p k f", p=128))
            s_W2 = pc.tile([DOUT, DOUT], F16)
            nc.sync.dma_start(out=s_W2[:], in_=t_W2[:])
            s_W3 = pc.tile([DOUT, DIN], F16)
            nc.sync.dma_start(out=s_W3[:], in_=t_W3[:])
            s_p12 = pc.tile([64, 4], F32)
            nc.sync.dma_start(out=s_p12[:], in_=t_p12[:])
            s_p3 = pc.tile([128, 4], F32)
            nc.sync.dma_start(out=s_p3[:], in_=t_p3[:])

            s_aT = pc.tile([64, NP], F16)
            s_tabT = pc.tile([64, NP], F16)
            s_z3 = pc.tile([128, 2, NP], F32, tag="zz")
            s_z12 = pc.tile([64, NP], F32, tag="zz")
            s_hst = pc.tile([128, TILES, 2 * DOUT], F16)
            s_stat = pc.tile([128, 8], F32)
            nc.vector.memset(s_hst[:], 0.0)
            s_vec = pc.tile([128, 8], F32)

            def build_table(l):
                for t in range(TILES):
                    tr = slice(t * 128, (t + 1) * 128)
                    if l == 0:
                        ph = psB.tile([128, DOUT], F32, tag="ph")
                        phT = psB.tile([64, 128], F32, tag="phT")
                        xt = pw.tile([128, 2, 128], F32, tag="xt")
                        nc.sync.dma_start(
                            out=xt[:],
                            in_=t_xT[:, tr].rearrange("(k p) n -> p k n", p=128))
                        for k in range(2):
                            nc.tensor.matmul(
                                ph[:], xt[:, k, :], s_W1[:, k, :],
                                start=(k == 0), stop=(k == 1))
                        for k in range(2):
                            nc.tensor.matmul(
                                phT[:], s_W1[:, k, :], xt[:, k, :],
                                start=(k == 0), stop=(k == 1))
                        nc.vector.tensor_tensor(
                            s_tabT[:, tr], phT[:], s_dvr[:, tr],
                            mybir.AluOpType.mult)
                    elif l == 1:
                        ph = psB.tile([128, DOUT], F32, tag="ph")
                        phT = psB.tile([64, 128], F32, tag="phT")
                        nc.tensor.matmul(
                            ph[:], s_aT[:, tr], s_W2[:], start=True, stop=True)
                        nc.tensor.matmul(
                            phT[:], s_W2[:], s_aT[:, tr], start=True, stop=True)
                        nc.vector.tensor_tensor(
                            s_tabT[:, tr], phT[:], s_dvr[:, tr],
                            mybir.AluOpType.mult)
                    else:
                        ph = psB.tile([128, DOUT], F16, tag="ph")
                        nc.tensor.transpose(ph[:], s_aT[:, tr], s_idn[:])
                        nc.vector.tensor_tensor(
                            s_tabT[:, tr], s_aT[:, tr], s_dvr[:, tr],
                            mybir.AluOpType.mult)
                    nc.vector.tensor_scalar(
                        s_hst[:, t, 0:DOUT], ph[:], s_dvl[:, t:t + 1], None,
                        mybir.AluOpType.mult)
                    if t == TL - 1:
                        nc.sync.dma_start(
                            out=h_locA[l][:].rearrange(
                                "(t p) f -> p t f", p=128),
                            in_=s_hst[:, 0:TL, :])
                        if NOCC:
                            nc.sync.dma_start(out=h_tabA[l][0:SZ_LO, :],
                                              in_=h_locA[l][:])
                        else:
                            nc.gpsimd.collective_compute(
                                "AllGather", mybir.AluOpType.bypass,
                                replica_groups=RG,
                                ins=[h_locA[l][:].opt()],
                                outs=[h_tabA[l][:].opt()])
                nc.sync.dma_start(
                    out=h_locB[l][:].rearrange(
                        "(t p) f -> p t f", p=128),
                    in_=s_hst[:, TL:TILES, :])
                if NOCC:
                    nc.sync.dma_start(out=h_tabB[l][0:SZ_HI, :],
                                      in_=h_locB[l][:])
                else:
                    nc.gpsimd.collective_compute(
                        "AllGather", mybir.AluOpType.bypass, replica_groups=RG,
                        ins=[h_locB[l][:].opt()],
                        outs=[h_tabB[l][:].opt()])

            def aggregate(l):
                if NOAGG:
                    if l < 2:
                        nc.vector.memset(s_z12[:], 0.0)
                    else:
                        nc.vector.memset(s_z3[:], 0.0)
                    return
                cb0 = 0
                f0 = 0
                for bt in batches:
                    nb = len(bt)
                    n_lo, n_hi = nb * K_lo * 128, nb * K_hi * 128
                    CBb = nb * (K_lo + K_hi)
                    m16 = pw.tile([128, CB, 2 * DOUT], F16, tag="m16")
                    if NOGATHER:
                        nc.vector.memset(m16[:], 0.0)
                    else:
                        nc.gpsimd.dma_gather(
                            m16[:, 0:nb * K_lo, :], h_tabA[l][:, :],
                            s_idx[:, f0:f0 + n_lo // 16], n_lo, n_lo, 2 * DOUT,
                            single_packet=False, queue_num=0)
                        nc.gpsimd.dma_gather(
                            m16[:, nb * K_lo:CBb, :], h_tabB[l][:, :],
                            s_idx[:, f0 + n_lo // 16:f0 + (n_lo + n_hi) // 16],
                            n_hi, n_hi, 2 * DOUT,
                            single_packet=False, queue_num=1)
                    f0 += (n_lo + n_hi) // 16
                    sS = pw.tile([128, CB, 128], F16, tag="sS")
                    nc.vector.tensor_tensor(
                        sS[:, 0:CBb, :],
                        s_iota[:].unsqueeze(1).broadcast_to([128, CBb, 128]),
                        s_dst[:, cb0:cb0 + CBb].unsqueeze(2).broadcast_to(
                            [128, CBb, 128]),
                        mybir.AluOpType.is_equal)
                    for j, t in enumerate(bt):
                        pa = psA.tile([64, 128], F32, tag="pa")
                        ck = (list(range(j * K_lo, (j + 1) * K_lo))
                              + list(range(nb * K_lo + j * K_hi,
                                           nb * K_lo + (j + 1) * K_hi)))
                        for i, c in enumerate(ck):
                            nc.tensor.matmul(
                                pa[:], m16[:, c, 0:DOUT], sS[:, c, :],
                                start=(i == 0), stop=(i == len(ck) - 1))
                        tr = slice(t * 128, (t + 1) * 128)
                        if l < 2:
                            nc.vector.tensor_tensor(
                                s_z12[:, tr], pa[:], s_tabT[:, tr],
                                mybir.AluOpType.add)
                            nc.vector.tensor_tensor(
                                s_z12[:, tr], s_z12[:, tr], s_dvr[:, tr],
                                mybir.AluOpType.mult)
                        else:
                            ag = pw.tile([64, 128], F16, tag="ag")
                            nc.vector.tensor_tensor(
                                ag[:], pa[:], s_tabT[:, tr], mybir.AluOpType.add)
                            nc.vector.tensor_tensor(
                                ag[:], ag[:], s_dvr[:, tr], mybir.AluOpType.mult)
                            for hf in range(2):
                                p3p = psC.tile([128, 128], F32, tag="p3p")
                                nc.tensor.matmul(
                                    p3p[:], s_W3[:, hf * 128:(hf + 1) * 128],
                                    ag[:], start=True, stop=True)
                                nc.vector.tensor_copy(s_z3[:, hf, tr], p3p[:])
                    cb0 += CBb

            def bn_stats(l):
                if NOBN:
                    nc.vector.memset(s_stat[:], 0.0)
                    nc.vector.memset(s_vec[:], 1.0)
                    return
                nrows = strows[l]
                CH = 5 * 128
                NCH = (NP + CH - 1) // CH
                if l < 2:
                    sq = pw.tile([64, CH], F32, tag="sq")
                    pt = pw.tile([64, NCH], F32, tag="pt")
                    nc.vector.reduce_sum(
                        s_stat[0:64, 0:1], s_z12[:, :], axis=mybir.AxisListType.X)
                    for i in range(NCH):
                        w = min(CH, NP - i * CH)
                        nc.vector.tensor_tensor(
                            sq[:, 0:w], s_z12[:, i * CH:i * CH + w],
                            s_z12[:, i * CH:i * CH + w], mybir.AluOpType.mult)
                        nc.vector.reduce_sum(
                            pt[:, i:i + 1], sq[:, 0:w], axis=mybir.AxisListType.X)
                    nc.vector.reduce_sum(
                        s_stat[0:64, 1:2], pt[:], axis=mybir.AxisListType.X)
                    nc.vector.memset(s_stat[0:64, 2:4], 0.0)
                else:
                    sq = pw.tile([128, CH], F32, tag="sq3")
                    pt = pw.tile([128, NCH], F32, tag="pt3")
                    for hf in range(2):
                        nc.vector.reduce_sum(
                            s_stat[:, 2 * hf:2 * hf + 1], s_z3[:, hf, :],
                            axis=mybir.AxisListType.X)
                        for i in range(NCH):
                            w = min(CH, NP - i * CH)
                            nc.vector.tensor_tensor(
                                sq[:, 0:w], s_z3[:, hf, i * CH:i * CH + w],
                                s_z3[:, hf, i * CH:i * CH + w],
                                mybir.AluOpType.mult)
                            nc.vector.reduce_sum(
                                pt[:, i:i + 1], sq[:, 0:w],
                                axis=mybir.AxisListType.X)
                        nc.vector.reduce_sum(
                            s_stat[:, 2 * hf + 1:2 * hf + 2], pt[:],
                            axis=mybir.AxisListType.X)
                nc.sync.dma_start(out=st_in[l][:], in_=s_stat[0:nrows, 0:4])
                if NOCC:
                    nc.sync.dma_start(out=st_out[l][:], in_=st_in[l][:])
                else:
                    nc.gpsimd.collective_compute(
                        "AllReduce", mybir.AluOpType.add, replica_groups=RG,
                        ins=[st_in[l][:].opt()], outs=[st_out[l][:].opt()])
                nc.sync.dma_start(out=s_stat[0:nrows, 4:8], in_=st_out[l][:])
                invN = 1.0 / float(N)
                npair = 1 if l < 2 else 2
                for p in range(npair):
                    r = slice(0, nrows)
                    su = s_stat[r, 4 + 2 * p:5 + 2 * p]
                    s2 = s_stat[r, 5 + 2 * p:6 + 2 * p]
                    m = s_vec[r, 4:5]
                    nc.vector.tensor_scalar(m, su, invN, None, mybir.AluOpType.mult)
                    ex2 = s_vec[r, 5:6]
                    nc.vector.tensor_scalar(s2, s2, invN, None, mybir.AluOpType.mult)
                    nc.vector.tensor_tensor(ex2, m, m, mybir.AluOpType.mult)
                    nc.vector.tensor_tensor(ex2, s2, ex2, mybir.AluOpType.subtract)
                    sd = s_vec[r, 6:7]
                    nc.vector.tensor_scalar(ex2, ex2, float(EPS), None,
                                            mybir.AluOpType.add)
                    nc.scalar.activation(sd, ex2, mybir.ActivationFunctionType.Sqrt)
                    inv = s_vec[r, 7:8]
                    nc.vector.reciprocal(inv, sd)
                    if l < 2:
                        g = s_p12[:, 2 * l:2 * l + 1]
                        be = s_p12[:, 2 * l + 1:2 * l + 2]
                    else:
                        g = s_p3[:, p:p + 1]
                        be = s_p3[:, 2 + p:3 + p]
                    sc = s_vec[r, 2 * p:2 * p + 1]
                    sh = s_vec[r, 2 * p + 1:2 * p + 2]
                    nc.vector.tensor_tensor(sc, g, inv, mybir.AluOpType.mult)
                    nc.vector.tensor_tensor(sh, m, sc, mybir.AluOpType.mult)
                    nc.vector.tensor_tensor(sh, be, sh, mybir.AluOpType.subtract)

            stage = os.environ.get("GCN_STAGE", "full")
            if stage == "gdump":
                bt = batches[0]
                nb = len(bt)
                n_lo, n_hi = nb * K_lo * 128, nb * K_hi * 128
                CBb = nb * (K_lo + K_hi)
                o_dbg = nc.declare_dram_parameter(
                    "dbg", [128, CB, 2 * DOUT], F16, isOutput=True)
                build_table(0)
                m16 = pw.tile([128, CB, 2 * DOUT], F16, tag="m16")
                nc.vector.memset(m16[:], 0.0)
                nc.gpsimd.dma_gather(
                    m16[:, 0:nb * K_lo, :], h_tabA[0][:, :],
                    s_idx[:, 0:n_lo // 16], n_lo, n_lo, 2 * DOUT,
                    single_packet=False, queue_num=0)
                nc.gpsimd.dma_gather(
                    m16[:, nb * K_lo:CBb, :], h_tabB[0][:, :],
                    s_idx[:, n_lo // 16:(n_lo + n_hi) // 16], n_hi, n_hi,
                    2 * DOUT, single_packet=False, queue_num=1)
                nc.sync.dma_start(out=o_dbg[:], in_=m16[:])
                nc.gpsimd.dma_start(out=o_out[0][:, 0:TILES * DOUT],
                                    in_=s_hst[:, :, 0:DOUT])
            elif stage == "tabdump":
                o_dbg = nc.declare_dram_parameter(
                    "dbg", [NCORE * 992, 2 * DOUT], F16, isOutput=True)
                build_table(0)
                half = os.environ.get("GCN_DUMPHALF", "A")
                for c8 in range(NCORE):
                    if half == "A":
                        nc.sync.dma_start(
                            out=o_dbg[c8 * 992:(c8 + 1) * 992, :],
                            in_=h_tabA[0][(c8 + 1) * SZ_LO - 992:(c8 + 1) * SZ_LO, :])
                    else:
                        nc.sync.dma_start(
                            out=o_dbg[c8 * 992:(c8 + 1) * 992, :],
                            in_=h_tabB[0][(c8 + 1) * SZ_HI - 992:(c8 + 1) * SZ_HI, :])
                nc.gpsimd.dma_start(out=o_out[0][:, 0:TILES * DOUT],
                                    in_=s_hst[:, :, 0:DOUT])
            elif stage == "tableng":
                # table build without the collective (timing control)
                for t in range(TILES):
                    tr = slice(t * 128, (t + 1) * 128)
                    ph = psB.tile([128, DOUT], F32, tag="ph")
                    xt = pw.tile([128, 2, 128], F32, tag="xt")
                    nc.sync.dma_start(
                        out=xt[:],
                        in_=t_xT[:, tr].rearrange("(k p) n -> p k n", p=128))
                    for k in range(2):
                        nc.tensor.matmul(ph[:], xt[:, k, :], s_W1[:, k, :],
                                         start=(k == 0), stop=(k == 1))
                    nc.vector.tensor_scalar(
                        s_hst[:, t, 0:DOUT], ph[:], s_dvl[:, t:t + 1], None,
                        mybir.AluOpType.mult)
                nc.sync.dma_start(
                    out=h_locA[0][:].rearrange("(t p) f -> p t f", p=128),
                    in_=s_hst[:, 0:TL, :])
                nc.gpsimd.dma_start(out=o_out[0][:, 0:TILES * DOUT],
                                    in_=s_hst[:, :, 0:DOUT])
            elif stage == "gonly":
                # table + collective + gathers only (no cast/S/matmul)
                build_table(0)
                f0 = 0
                cb0 = 0
                for bt in batches:
                    nb = len(bt)
                    n_lo, n_hi = nb * K_lo * 128, nb * K_hi * 128
                    CBb = nb * (K_lo + K_hi)
                    m32 = pg.tile([128, CB, 2 * DOUT], F16, tag="m32")
                    nc.gpsimd.dma_gather(
                        m32[:, 0:nb * K_lo, :], h_tabA[0][:, :],
                        s_idx[:, f0:f0 + n_lo // 16], n_lo, n_lo, 2 * DOUT,
                        single_packet=False)
                    f0 += n_lo // 16
                    nc.gpsimd.dma_gather(
                        m32[:, nb * K_lo:CBb, :], h_tabB[0][:, :],
                        s_idx[:, f0:f0 + n_hi // 16], n_hi, n_hi, 2 * DOUT,
                        single_packet=False)
                    f0 += n_hi // 16
                    cb0 += CBb
                nc.gpsimd.dma_start(out=o_out[0][:, 0:TILES * DOUT],
                                    in_=s_hst[:, :, 0:DOUT])
            elif stage == "table":
                build_table(0)
                nc.gpsimd.dma_start(out=o_out[0][:, 0:TILES * DOUT],
                                    in_=s_hst[:, :, 0:DOUT])
                nc.vector.memset(s_z3[:, 1, 0:128], 0.0)
                nc.gpsimd.dma_start(out=o_out[1][:, 0:128], in_=s_z3[:, 1, 0:128])
            elif stage == "agg":
                build_table(0)
                aggregate(0)
                nc.sync.dma_start(out=o_out[0][0:64, :], in_=s_z12[:, :])
                nc.sync.dma_start(out=o_out[1][0:64, :], in_=s_z12[:, :])
            if stage == "full":
                for l in range(2):
                    build_table(l)
                    aggregate(l)
                    bn_stats(l)
                    nc.scalar.activation(
                        s_aT[:], s_z12[:, :],
                        mybir.ActivationFunctionType.Relu,
                        bias=s_vec[0:64, 1:2], scale=s_vec[0:64, 0:1])
                build_table(2)
                aggregate(2)
                bn_stats(2)
                RCH = 4 * 128
                for hf in range(2):
                    nc.scalar.activation(
                        s_z3[:, hf, :], s_z3[:, hf, :],
                        mybir.ActivationFunctionType.Identity,
                        bias=s_vec[:, 2 * hf + 1:2 * hf + 2],
                        scale=s_vec[:, 2 * hf:2 * hf + 1])
                    for i in range((NP + RCH - 1) // RCH):
                        w = min(RCH, NP - i * RCH)
                        rs = slice(i * RCH, i * RCH + w)
                        xt = pw.tile([128, RCH], F32, tag="xr")
                        nc.sync.dma_start(
                            out=xt[:, 0:w], in_=t_xT[hf * 128:(hf + 1) * 128, rs])
                        nc.vector.tensor_tensor(
                            s_z3[:, hf, rs], s_z3[:, hf, rs], xt[:, 0:w],
                            mybir.AluOpType.add)
                        nc.vector.tensor_scalar(
                            s_z3[:, hf, rs], s_z3[:, hf, rs], 0.0, None,
                            mybir.AluOpType.max)
                        nc.sync.dma_start(out=o_out[hf][:, rs],
                                          in_=s_z3[:, hf, rs])

    nc.finalize()
    return nc


# ----------------------------------------------------------------------------
# entry point
# ----------------------------------------------------------------------------

def _prepare(x, ei, W1, g1, be1, W2, g2, be2, W3, g3, be3):
    N, NP, TILES = _CFG["N"], _CFG["NP"], _CFG["TILES"]
    x = np.asarray(x, np.float32)
    ei = np.asarray(ei, np.int32)
    src, dst, dinv, core_of, tile_of, col_of, loc_of, pid_of = _prep_graph(ei)
    per_core, meta = _build_streams(src, dst, core_of, tile_of, col_of, loc_of)

    iota = np.tile(np.arange(128, dtype=np.float16)[None, :], (128, 1))
    idn = np.eye(64, dtype=np.float16)
    p12 = np.stack([np.asarray(g1), np.asarray(be1),
                    np.asarray(g2), np.asarray(be2)], axis=1).astype(np.float32)
    g3c = np.asarray(g3, np.float32).reshape(2, 128).T
    be3c = np.asarray(be3, np.float32).reshape(2, 128).T
    p3 = np.concatenate([g3c, be3c], axis=1).astype(np.float32)

    in_maps = []
    for c in range(NCORE):
        nodes_c = np.nonzero(core_of == c)[0]
        lidx = loc_of[nodes_c]
        xT = np.zeros((DIN, NP), np.float32)
        xT[:, lidx] = x[nodes_c].T
        dvr = np.zeros((NP,), np.float32)
        dvr[lidx] = dinv[nodes_c]
        dvl = dvr.reshape(TILES, 128).T.copy()
        idx_all, dst_all = per_core[c]
        in_maps.append({
            "xT": xT, "idx": np.ascontiguousarray(idx_all),
            "dst": np.ascontiguousarray(dst_all), "iota": iota, "idn": idn,
            "dvr": np.tile(dvr[None, :], (64, 1)).astype(np.float16),
            "dvl": np.ascontiguousarray(dvl),
            "W1": np.asarray(W1, np.float32),
            "W2": np.asarray(W2, np.float32).astype(np.float16),
            "W3": np.asarray(W3, np.float32).astype(np.float16),
            "p12": p12, "p3": p3,
        })
    return in_maps, meta, core_of, loc_of


def kernel(x, ei, batch, W1, b1, g1, be1, W2, b2, g2, be2, W3, b3, g3, be3):
    global LAST_EXEC_NS
    from concourse.bass_utils import run_bass_kernel_spmd

    N, NP = _CFG["N"], _CFG["NP"]
    in_maps, meta, core_of, loc_of = _prepare(
        x, ei, W1, g1, be1, W2, g2, be2, W3, g3, be3)
    nc = _build_nc(meta, in_maps[0]["idx"].shape[1], in_maps[0]["dst"].shape[1])

    trace = bool(int(os.environ.get("GCN_TRACE", "0")))
    res = run_bass_kernel_spmd(nc, in_maps, list(range(NCORE)), trace=trace)
    if res.exec_time_ns is not None:
        LAST_EXEC_NS = res.exec_time_ns

    out = np.empty((N, DIN), np.float32)
    for c in range(NCORE):
        nodes_c = np.nonzero(core_of == c)[0]
        arr = res.results[c]["outT"].reshape(DIN, NP)
        out[nodes_c] = arr[:, loc_of[nodes_c]].T
    return out

